# revision 1
# baseline (speedup 1.0000x reference)
# Trainium2 Bass kernel for nn_AttnModel_64098091926054.
#
# Strategy: pure data parallel over batch (256 boards -> 32 per core x 8 cores).
# Host-side constant folding shrinks device weight traffic ~4x:
#   - softmax over cells is shift-invariant, so the x-dependent k-term
#     (x @ kvx_w[:, :D]) cancels -> never computed.
#   - attention logits: dots[b,p] = bfeat[b,p,:19] . g[b], with
#     g = x @ (q_w @ Wk^T)/sqrt(D) -- q_w (512x512) folds to qk_w (512x19).
#   - value path: vals @ fin_w distributes: x @ (kvx_v @ fin_w) +
#     s @ (Wv @ fin_w) + const -- removes kvx_v as a separate matmul.
# Remaining big weights per layer: afin, fc0, fc1 (each 512x512) shipped bf16
# in one 1.5MB DMA per layer. Activations: residual kept f32 as xT (128,4*32);
# bf16 copies feed the PE. Attention math f32 on DVE with the 81 cells split
# 3x27 so (batch 32 x 3) = 96 partitions stay busy. |alpha| folds into the
# ACT relu scale; the residual update is one add/sub per (128,128) block.

import numpy as np
import ml_dtypes

import concourse.bass as bass
import concourse.bacc as bacc
import concourse.mybir as mybir
import concourse.tile as tile
from concourse.bass_utils import run_bass_kernel_spmd
from concourse.masks import make_identity

BS, D, L, B, P, POSD, J = 9, 512, 8, 256, 81, 12, 19
NCORES = 8
NB = B // NCORES          # 32 boards per core
P3, PQ = 3, 27            # 81 cells = 3 partition-groups x 27
NP = NB * P3              # 96 partitions for attention math
OFFSETS = [(-1, 0), (-1, 1), (0, -1), (0, 0), (0, 1), (-1, -1), (-1, 0)]

f32 = mybir.dt.float32
bf16 = mybir.dt.bfloat16
bf16_np = ml_dtypes.bfloat16

# const-pack column offsets (all f32, one DMA): bfpj | bfjp | e2 | e2t | wpost
C_BFPJ = 0
C_BFJP = C_BFPJ + PQ * J          # 513
C_E2 = C_BFJP + PQ * J            # 1026
C_E2T = C_E2 + NB                 # 1058
C_POST = C_E2T + NP               # 1154
C_END = C_POST + P                # 1235
WB_F = 12 * D + 4 * J             # 6220 = [afin|fc0|fc1] k-tiled + qk_w


def _positions():
    lin = np.linspace(0.0, 1.0, BS, dtype=np.float32)
    rs, cs = np.meshgrid(lin, lin, indexing="ij")
    zs = (rs + cs) / 2.0
    xs = np.stack([rs, cs, zs], -1).astype(np.float32)
    feats = []
    for p in [4.0 / (BS - 1), 16.0 / (BS - 1)]:
        a = (2.0 * np.pi * xs / p).astype(np.float32)
        feats.append(np.concatenate([np.cos(a), np.sin(a)], -1).astype(np.float32))
    return np.concatenate(feats, -1)  # (9, 9, 12)


def _prepare(obs, pos):
    single = obs[..., 0] - obs[..., 1]
    aug = np.pad(single, ((0, 0), (1, 1), (1, 1)))
    w = aug.shape[-1]
    outs = [aug[:, 1 + r : w - 1 + r, 1 + c : w - 1 + c] for (r, c) in OFFSETS]
    neigh = np.stack(outs, -1)
    n = obs.shape[0]
    stack = np.concatenate(
        [neigh, np.broadcast_to(pos, (n,) + pos.shape)], -1
    ).astype(np.float32)
    return stack.reshape(n, P, J)  # (B, 81, 19)


def _fold(inp):
    """Host-side constant folding of weights. All f32 numpy."""
    scale = np.float32(1.0 / np.sqrt(D))
    Wk = inp["kvb_w"][:, :, :D]                                   # (L,19,512)
    Wv = inp["kvb_w"][:, :, D:]
    kvx_v = inp["kvx_w"][:, :, D:]                                # (L,512,512)
    qk_w = np.einsum("ldh,ljh->ldj", inp["q_w"], Wk) * scale      # (L,512,19)
    qk_b = np.einsum("lh,ljh->lj", inp["q_b"], Wk) * scale        # (L,19)
    afin = np.einsum("lde,leh->ldh", kvx_v, inp["fin_w"])         # (L,512,512)
    sfin = np.einsum("lje,leh->ljh", Wv, inp["fin_w"])            # (L,19,512)
    bias_v = inp["kvx_b"][:, D:] + inp["kvb_b"][:, D:]
    cfin = np.einsum("le,leh->lh", bias_v, inp["fin_w"]) + inp["fin_b"]
    return qk_w, qk_b, afin, sfin, cfin


def _build_nc(alpha):
    nc = bacc.Bacc("TRN2", target_bir_lowering=False, debug=False)

    d_cpack = nc.dram_tensor("cpack", [128, C_END], f32, kind="ExternalInput")
    d_wbig = nc.dram_tensor("wbig", [L, 128, WB_F], bf16, kind="ExternalInput")
    d_wsfin = nc.dram_tensor("wsfin", [J, L * D], bf16, kind="ExternalInput")
    d_wbias = nc.dram_tensor("wbias", [1, L * (3 * D + J)], f32, kind="ExternalInput")
    d_whead = nc.dram_tensor("whead", [128, 4 * POSD], bf16, kind="ExternalInput")
    d_out = nc.dram_tensor("out", [NB, P], f32, kind="ExternalOutput")

    def bcast_mid(ap2d, n):
        # (p, k) AP -> (p, n, k) with step-0 broadcast in the middle
        return bass.AP(
            tensor=ap2d.tensor, offset=ap2d.offset,
            ap=[ap2d.ap[0], [0, n], ap2d.ap[1]],
        )

    with tile.TileContext(nc) as tc:
        with (
            tc.tile_pool(name="consts", bufs=1) as consts,
            tc.tile_pool(name="wpool", bufs=8) as wpool,
            tc.tile_pool(name="ap", bufs=3) as apool,
            tc.tile_pool(name="pm", bufs=2, space="PSUM") as pm,
            tc.tile_pool(name="pt", bufs=4, space="PSUM") as pt,
            tc.tile_pool(name="px", bufs=2, space="PSUM") as px,
        ):
            # constants (4 DMAs total)
            cpack = consts.tile([128, C_END], f32)
            nc.sync.dma_start(out=cpack, in_=d_cpack[:, :])
            sfall = consts.tile([J, L * D], bf16)
            nc.sync.dma_start(out=sfall, in_=d_wsfin[:, :])
            ball = consts.tile([1, L * (3 * D + J)], f32)
            nc.sync.dma_start(out=ball, in_=d_wbias[:, :])
            whead = consts.tile([128, 4 * POSD], bf16)
            nc.sync.dma_start(out=whead, in_=d_whead[:, :])
            ident = consts.tile([NB, NB], f32)
            make_identity(nc, ident[:, :])
            ones1 = consts.tile([1, NB], f32)
            nc.vector.memset(ones1, 1.0)

            bfpj3 = cpack[:NP, C_BFPJ:C_BFJP].rearrange("p (a b) -> p a b", b=J)
            bfjp3 = cpack[:NP, C_BFJP:C_E2].rearrange("p (a b) -> p a b", b=PQ)
            e2 = cpack[:NP, C_E2:C_E2T]        # (96, 32)
            e2t = cpack[:NB, C_E2T:C_POST]     # (32, 96)
            wpost = cpack[:POSD, C_POST:C_END]  # (12, 81)

            # residual stream: xT (128, 4*32) f32 + bf16 copy; free-slice kt
            # holds rows 128*kt..128*kt+127 of x^T.
            xT_f = apool.tile([128, 4 * NB], f32, tag="xf")
            nc.vector.memset(xT_f, 0.0)
            xT_b = apool.tile([128, 4 * NB], bf16, tag="xb")
            nc.vector.memset(xT_b, 0.0)

            def xslice(t, kt):
                return t[:, kt * NB : (kt + 1) * NB]

            def residual_update(v_sb, al):
                # x_new = x + al * v  (|al| already folded into v_sb)
                nonlocal xT_f, xT_b
                vT = px.tile([128, 4 * NB], f32, tag="tpx")
                for kt in range(4):
                    nc.tensor.transpose(
                        xslice(vT, kt), v_sb[:, kt * 128 : (kt + 1) * 128], ident
                    )
                nf = apool.tile([128, 4 * NB], f32, tag="xf")
                op = mybir.AluOpType.add if al >= 0 else mybir.AluOpType.subtract
                nc.vector.tensor_tensor(nf, xT_f, vT, op=op)
                nb_ = apool.tile([128, 4 * NB], bf16, tag="xb")
                nc.vector.tensor_copy(nb_, nf)
                xT_f, xT_b = nf, nb_

            for l in range(L):
                wb = wpool.tile([128, WB_F], bf16, tag="wb")
                nc.sync.dma_start(out=wb, in_=d_wbig[l, :, :])
                ws = sfall[:, l * D : (l + 1) * D]
                bias0 = l * (3 * D + J)

                # ---- attention: g = x @ qk_w + qk_b  (32,19) ----
                g_ps = pt.tile([NB, J], f32, tag="tp")
                for kt in range(4):
                    nc.tensor.matmul(
                        g_ps, xslice(xT_b, kt),
                        wb[:, 12 * D + kt * J : 12 * D + (kt + 1) * J],
                        start=(kt == 0), stop=False,
                    )
                nc.tensor.matmul(
                    g_ps, ones1, ball[:, bias0 + 3 * D : bias0 + 3 * D + J],
                    start=False, stop=True,
                )
                g_sb = apool.tile([NB, J], f32, tag="g")
                nc.vector.tensor_copy(g_sb, g_ps)
                # replicate per-board g to 96 partitions: g3 = e2t.T @ g
                g3_ps = pt.tile([NP, J], f32, tag="tp")
                nc.tensor.matmul(g3_ps, e2t, g_sb, start=True, stop=True)
                g3 = apool.tile([NP, J], f32, tag="g3")
                nc.vector.tensor_copy(g3, g3_ps)

                # dots[b,p] = bfeat . g  -> (96, 27)
                t1 = apool.tile([NP, PQ * J], f32, tag="t1")
                t1_3 = t1[:, :].rearrange("p (a b) -> p a b", b=J)
                nc.vector.tensor_tensor(
                    t1_3, bfpj3, bcast_mid(g3[:, :], PQ), op=mybir.AluOpType.mult
                )
                dots = apool.tile([NP, PQ], f32, tag="dots")
                nc.vector.tensor_reduce(
                    dots, t1_3, axis=mybir.AxisListType.X, op=mybir.AluOpType.add
                )
                # e = exp(dots) (|dots| < ~1 so no max-shift needed), rowsum fused
                e4 = apool.tile([NP, PQ], f32, tag="e4")
                rs = apool.tile([NP, 1], f32, tag="rs")
                nc.scalar.activation(
                    e4, dots, mybir.ActivationFunctionType.Exp, accum_out=rs
                )
                # s_un[b,j] = sum_p e[b,p] bfeat[b,p,j]
                t2 = apool.tile([NP, J * PQ], f32, tag="t2")
                t2_3 = t2[:, :].rearrange("p (a b) -> p a b", b=PQ)
                nc.vector.tensor_tensor(
                    t2_3, bfjp3, bcast_mid(e4[:, :], J), op=mybir.AluOpType.mult
                )
                s4 = apool.tile([NP, J], f32, tag="s4")
                nc.vector.tensor_reduce(
                    s4, t2_3, axis=mybir.AxisListType.X, op=mybir.AluOpType.add
                )
                # group-sum the 3 cell-blocks per board: (32, 19) and (32, 1)
                grp = pt.tile([NB, J + 1], f32, tag="tp")
                nc.tensor.matmul(grp[:, 0:J], e2, s4, start=True, stop=True)
                nc.tensor.matmul(grp[:, J : J + 1], e2, rs, start=True, stop=True)
                recip = apool.tile([NB, 1], f32, tag="rc")
                nc.vector.reciprocal(recip, grp[:, J : J + 1])
                s_sb = apool.tile([NB, J], f32, tag="s")
                nc.vector.tensor_scalar_mul(s_sb, grp[:, 0:J], recip)
                sT_ps = pt.tile([J, NB], f32, tag="tp")
                nc.tensor.transpose(sT_ps, s_sb, ident)
                sT = apool.tile([J, NB], bf16, tag="sT")
                nc.vector.tensor_copy(sT, sT_ps)

                al = float(alpha[l])
                aab = abs(al)

                # ---- t = relu(x@afin + s@sfin + cfin), scaled by |alpha| ----
                t_ps = pm.tile([NB, D], f32, tag="mm")
                for kt in range(4):
                    nc.tensor.matmul(
                        t_ps, xslice(xT_b, kt), wb[:, kt * D : (kt + 1) * D],
                        start=(kt == 0), stop=False,
                    )
                nc.tensor.matmul(
                    t_ps, ones1, ball[:, bias0 : bias0 + D], start=False, stop=False
                )
                nc.tensor.matmul(t_ps, sT, ws, start=False, stop=True)
                t_sb = apool.tile([NB, D], f32, tag="act")
                nc.scalar.activation(
                    t_sb, t_ps, mybir.ActivationFunctionType.Relu, scale=aab
                )
                residual_update(t_sb, al)

                # ---- u = relu(x@fc0 + fc0_b) ----
                u_ps = pm.tile([NB, D], f32, tag="mm")
                for kt in range(4):
                    nc.tensor.matmul(
                        u_ps, xslice(xT_b, kt), wb[:, (4 + kt) * D : (5 + kt) * D],
                        start=(kt == 0), stop=False,
                    )
                nc.tensor.matmul(
                    u_ps, ones1, ball[:, bias0 + D : bias0 + 2 * D],
                    start=False, stop=True,
                )
                u_sb = apool.tile([NB, D], f32, tag="act")
                nc.scalar.activation(u_sb, u_ps, mybir.ActivationFunctionType.Relu)
                uT_ps = px.tile([128, 4 * NB], f32, tag="tpx")
                for kt in range(4):
                    nc.tensor.transpose(
                        xslice(uT_ps, kt), u_sb[:, kt * 128 : (kt + 1) * 128], ident
                    )
                uT_b = apool.tile([128, 4 * NB], bf16, tag="uT")
                nc.vector.tensor_copy(uT_b, uT_ps)

                # ---- y = (u@fc1 + fc1_b) scaled by |alpha| ----
                y_ps = pm.tile([NB, D], f32, tag="mm")
                for kt in range(4):
                    nc.tensor.matmul(
                        y_ps, xslice(uT_b, kt), wb[:, (8 + kt) * D : (9 + kt) * D],
                        start=(kt == 0), stop=False,
                    )
                nc.tensor.matmul(
                    y_ps, ones1, ball[:, bias0 + 2 * D : bias0 + 3 * D],
                    start=False, stop=True,
                )
                y_sb = apool.tile([NB, D], f32, tag="act")
                nc.scalar.activation(
                    y_sb, y_ps, mybir.ActivationFunctionType.Copy, scale=aab
                )
                residual_update(y_sb, al)

            # ---- head: logits = log_softmax((x @ head_w) @ posT) ----
            z_ps = pt.tile([NB, POSD], f32, tag="tp")
            for kt in range(4):
                nc.tensor.matmul(
                    z_ps, xslice(xT_b, kt), whead[:, kt * POSD : (kt + 1) * POSD],
                    start=(kt == 0), stop=(kt == 3),
                )
            z_sb = apool.tile([NB, POSD], f32, tag="z")
            nc.scalar.activation(z_sb, z_ps, mybir.ActivationFunctionType.Copy)
            zT_ps = pt.tile([POSD, NB], f32, tag="tp")
            nc.tensor.transpose(zT_ps, z_sb, ident)
            zT = apool.tile([POSD, NB], f32, tag="zT")
            nc.scalar.activation(zT, zT_ps, mybir.ActivationFunctionType.Copy)
            lg_ps = pt.tile([NB, P], f32, tag="tp")
            nc.tensor.matmul(lg_ps, zT, wpost, start=True, stop=True)
            lg = apool.tile([NB, P], f32, tag="lg")
            nc.scalar.activation(lg, lg_ps, mybir.ActivationFunctionType.Copy)
            mx = apool.tile([NB, 1], f32, tag="mx")
            nc.vector.tensor_reduce(
                mx, lg[:, :], axis=mybir.AxisListType.X, op=mybir.AluOpType.max
            )
            negmx = apool.tile([NB, 1], f32, tag="nmx")
            nc.vector.tensor_scalar_mul(negmx, mx, -1.0)
            ex = apool.tile([NB, P], f32, tag="ex")
            sume = apool.tile([NB, 1], f32, tag="sume")
            nc.scalar.activation(
                ex, lg, mybir.ActivationFunctionType.Exp,
                bias=negmx[:, :], accum_out=sume,
            )
            lse = apool.tile([NB, 1], f32, tag="lse")
            nc.scalar.activation(lse, sume, mybir.ActivationFunctionType.Ln)
            c = apool.tile([NB, 1], f32, tag="c")
            nc.vector.tensor_add(c, mx, lse)
            outf = apool.tile([NB, P], f32, tag="outf")
            nc.vector.tensor_scalar(
                outf, lg[:, :], c[:, :], None, op0=mybir.AluOpType.subtract
            )
            nc.sync.dma_start(out=d_out[:, :], in_=outf)

    nc.finalize()
    return nc


def kernel(**inputs):
    inp = {k: np.asarray(v, dtype=np.float32) for k, v in inputs.items()}
    pos = _positions()
    bfeat = _prepare(inp["obs"], pos)  # (256, 81, 19)
    qk_w, qk_b, afin, sfin, cfin = _fold(inp)

    # big per-layer weights, k-tiled into SBUF layout, + qk_w columns
    big = np.stack([afin, inp["fc0_w"], inp["fc1_w"]], 1)  # (L,3,512,512)
    wbig_main = (
        big.reshape(L, 3, 4, 128, D).transpose(0, 3, 1, 2, 4).reshape(L, 128, 12 * D)
    )
    wqk = qk_w.reshape(L, 4, 128, J).transpose(0, 2, 1, 3).reshape(L, 128, 4 * J)
    wbig = np.concatenate([wbig_main, wqk], axis=2).astype(bf16_np)  # (L,128,6220)

    wsfin = np.ascontiguousarray(
        sfin.transpose(1, 0, 2).reshape(J, L * D)
    ).astype(bf16_np)
    wbias = np.concatenate(
        [cfin, inp["fc0_b"], inp["fc1_b"], qk_b], axis=1
    ).reshape(1, L * (3 * D + J)).astype(np.float32)
    whead = (
        inp["head_w"].reshape(4, 128, POSD).transpose(1, 0, 2)
        .reshape(128, 4 * POSD).astype(bf16_np)
    )

    e2 = np.zeros((NP, NB), np.float32)   # group-sum: e2[b*3+p3, b] = 1
    for b in range(NB):
        for p3 in range(P3):
            e2[b * P3 + p3, b] = 1.0

    in_maps = []
    for c in range(NCORES):
        bf = bfeat[c * NB : (c + 1) * NB]          # (32, 81, 19)
        bf3 = bf.reshape(NB, P3, PQ, J)
        cpack = np.zeros((128, C_END), np.float32)
        cpack[:NP, C_BFPJ:C_BFJP] = bf3.reshape(NP, PQ * J)
        cpack[:NP, C_BFJP:C_E2] = np.ascontiguousarray(
            bf3.transpose(0, 1, 3, 2)
        ).reshape(NP, J * PQ)
        cpack[:NP, C_E2:C_E2T] = e2
        cpack[:NB, C_E2T:C_POST] = e2.T
        cpack[:POSD, C_POST:C_END] = pos.reshape(P, POSD).T
        in_maps.append({
            "cpack": cpack,
            "wbig": wbig, "wsfin": wsfin, "wbias": wbias, "whead": whead,
        })

    nc = _build_nc([float(a) for a in inp["alpha"]])
    res = run_bass_kernel_spmd(nc, in_maps, core_ids=list(range(NCORES)))
    out = np.concatenate([r["out"] for r in res.results], axis=0)  # (256, 81)
    return out.astype(np.float32)



# revision 6
# speedup vs baseline: 1.6291x; 1.6291x over previous
# Trainium2 Bass kernel for nn_AttnModel_64098091926054.
#
# Strategy: pure data parallel over batch (256 boards -> 32 per core x 8 cores).
# Host-side constant folding (softmax shift-invariance kills the x-dependent
# k-term; q_w folds into qk_w (512x19); kvx_v/Wv fold through fin_w).
#
# v2: transposed compute layout. The residual lives as x^T (128 part, 4x32)
# and the three big per-layer matmuls keep the WEIGHT stationary (lhsT =
# natural (k,m) weight tiles, fp8e4 for 4x fast-weight-load) while the skinny
# activations (128, 32) stream as rhs in bf16. This makes every bias
# per-partition (free via ACT bias / fused DVE scalar_tensor_tensor), removes
# all transposes, and is immune to the PE HAM clock gate (LDWEIGHTS runs at
# a fixed 1.2 GHz x FWL regardless of warm-up). fin bias rides inside the
# K=20 sfin-augmented matmul; alpha is applied in the fused relu/residual DVE
# ops so the fp8 weights stay in the normal range. Attention math runs on DVE
# in bf16 with inner dims padded to even lengths for the 2x/4x perf modes.

import numpy as np
import ml_dtypes

import concourse.bass as bass
import concourse.bacc as bacc
import concourse.mybir as mybir
import concourse.tile as tile
from concourse.bass_utils import run_bass_kernel_spmd

BS, D, L, B, P, POSD, J = 9, 512, 8, 256, 81, 12, 19
NCORES = 8
NB = B // NCORES          # 32 boards per core
P3, PQ = 3, 27            # 81 cells = 3 partition-groups x 27
NP = NB * P3              # 96 partitions for attention math
JP = J + 1                # 20: j padded to even for DVE 2x mode
PQP = PQ + 1              # 28: pq padded to even
OFFSETS = [(-1, 0), (-1, 1), (0, -1), (0, 0), (0, 1), (-1, -1), (-1, 0)]

f32 = mybir.dt.float32
bf16 = mybir.dt.bfloat16
fp8 = mybir.dt.float8e4
bf16_np = ml_dtypes.bfloat16
fp8_np = ml_dtypes.float8_e4m3

# cpk16 column offsets (bf16): bfpj (27x20) | bfjp (19x28) | wpost (12,81)
C_BFPJ = 0
C_BFJP = C_BFPJ + PQ * JP          # 540
C_POST = C_BFJP + J * PQP          # 1072
C16_END = C_POST + P               # 1153
AluOp = mybir.AluOpType
Act = mybir.ActivationFunctionType


def _positions():
    lin = np.linspace(0.0, 1.0, BS, dtype=np.float32)
    rs, cs = np.meshgrid(lin, lin, indexing="ij")
    zs = (rs + cs) / 2.0
    xs = np.stack([rs, cs, zs], -1).astype(np.float32)
    feats = []
    for p in [4.0 / (BS - 1), 16.0 / (BS - 1)]:
        a = (2.0 * np.pi * xs / p).astype(np.float32)
        feats.append(np.concatenate([np.cos(a), np.sin(a)], -1).astype(np.float32))
    return np.concatenate(feats, -1)  # (9, 9, 12)


def _prepare(obs, pos):
    single = obs[..., 0] - obs[..., 1]
    aug = np.pad(single, ((0, 0), (1, 1), (1, 1)))
    w = aug.shape[-1]
    outs = [aug[:, 1 + r : w - 1 + r, 1 + c : w - 1 + c] for (r, c) in OFFSETS]
    neigh = np.stack(outs, -1)
    n = obs.shape[0]
    stack = np.concatenate(
        [neigh, np.broadcast_to(pos, (n,) + pos.shape)], -1
    ).astype(np.float32)
    return stack.reshape(n, P, J)  # (B, 81, 19)


def _fold(inp):
    """Host-side constant folding of weights. All f32 numpy, unscaled."""
    scale = np.float32(1.0 / np.sqrt(D))
    Wk = inp["kvb_w"][:, :, :D]                                   # (L,19,512)
    Wv = inp["kvb_w"][:, :, D:]
    kvx_v = inp["kvx_w"][:, :, D:]                                # (L,512,512)
    qk_w = np.einsum("ldh,ljh->ldj", inp["q_w"], Wk) * scale      # (L,512,19)
    qk_b = np.einsum("lh,ljh->lj", inp["q_b"], Wk) * scale        # (L,19)
    afin = np.einsum("lde,leh->ldh", kvx_v, inp["fin_w"])         # (L,512,512)
    sfin = np.einsum("lje,leh->ljh", Wv, inp["fin_w"])            # (L,19,512)
    bias_v = inp["kvx_b"][:, D:] + inp["kvb_b"][:, D:]
    cfin = np.einsum("le,leh->lh", bias_v, inp["fin_w"]) + inp["fin_b"]
    return qk_w, qk_b, afin, sfin, cfin


def _ktile_lhsT(W):
    """(L,512,512) -> (L,128, 16*128) with col ((o*4+kt)*128+m) = W[l,kt*128+k,o*128+m]."""
    Lx = W.shape[0]
    return np.ascontiguousarray(
        W.reshape(Lx, 4, 128, 4, 128).transpose(0, 2, 3, 1, 4).reshape(Lx, 128, 2048)
    )


def _build_nc(alpha):
    nc = bacc.Bacc("TRN2", target_bir_lowering=False, debug=False)

    d_cpk16 = nc.dram_tensor("cpk16", [NP, C16_END], bf16, kind="ExternalInput")
    d_cpk32 = nc.dram_tensor("cpk32", [NP, 128], f32, kind="ExternalInput")
    d_wbig = nc.dram_tensor("wbig", [L, 128, 3 * 2048], fp8, kind="ExternalInput")
    d_qkall = nc.dram_tensor("qkall", [128, L * 4 * J], bf16, kind="ExternalInput")
    d_sfall = nc.dram_tensor("sfall", [JP, L * D], bf16, kind="ExternalInput")
    d_b01 = nc.dram_tensor("b01", [128, L * 9], f32, kind="ExternalInput")
    d_qb = nc.dram_tensor("qb", [1, L * J], f32, kind="ExternalInput")
    d_whead = nc.dram_tensor("whead", [128, 4 * POSD], bf16, kind="ExternalInput")
    d_out = nc.dram_tensor("out", [NB, P], f32, kind="ExternalOutput")

    def bcast_mid(ap2d, n):
        # (p, k) AP -> (p, n, k) with step-0 broadcast in the middle
        return bass.AP(
            tensor=ap2d.tensor, offset=ap2d.offset,
            ap=[ap2d.ap[0], [0, n], ap2d.ap[1]],
        )

    def bcast_free(ap_col, n):
        # (p, 1) AP -> (p, n) with step-0 broadcast along free dim
        return bass.AP(
            tensor=ap_col.tensor, offset=ap_col.offset,
            ap=[ap_col.ap[0], [0, n]],
        )

    with tile.TileContext(nc) as tc:
        with (
            tc.tile_pool(name="consts", bufs=1) as consts,
            tc.tile_pool(name="wpool", bufs=8) as wpool,
            tc.tile_pool(name="ap", bufs=3) as apool,
            tc.tile_pool(name="attn", bufs=1) as atp,
            tc.tile_pool(name="pm", bufs=3, space="PSUM") as pm,
            tc.tile_pool(name="pt", bufs=2, space="PSUM") as pt,
        ):
            # ---- constants (7 DMAs) ----
            cpk16 = consts.tile([NP, C16_END], bf16)
            nc.sync.dma_start(out=cpk16, in_=d_cpk16[:, :])
            cpk32 = consts.tile([NP, 128], f32)
            nc.sync.dma_start(out=cpk32, in_=d_cpk32[:, :])
            qkall = consts.tile([128, L * 4 * J], bf16)
            nc.sync.dma_start(out=qkall, in_=d_qkall[:, :])
            sfall = consts.tile([JP, L * D], bf16)
            nc.sync.dma_start(out=sfall, in_=d_sfall[:, :])
            b01 = consts.tile([128, L * 9], f32)
            nc.sync.dma_start(out=b01, in_=d_b01[:, :])
            qb = consts.tile([1, L * J], f32)
            nc.sync.dma_start(out=qb, in_=d_qb[:, :])
            whead = consts.tile([128, 4 * POSD], bf16)
            nc.sync.dma_start(out=whead, in_=d_whead[:, :])

            bfpj3 = cpk16[:NP, C_BFPJ:C_BFJP].rearrange("p (a b) -> p a b", b=JP)
            bfjp3 = cpk16[:NP, C_BFJP:C_POST].rearrange("p (a b) -> p a b", b=PQP)
            wpost = cpk16[:POSD, C_POST:C16_END]   # (12, 81) bf16
            e2 = cpk32[:NP, 0:NB]                  # (96, 32) f32
            e2t = cpk32[:NB, NB:128]               # (32, 96) f32

            ones1 = consts.tile([1, NB], f32)
            nc.vector.memset(ones1, 1.0)
            # persistent attention buffers (serial chain -> single-buffered)
            sT_buf = consts.tile([JP, NB], bf16)
            nc.vector.memset(sT_buf, 1.0)   # row 19 stays 1.0 (cfin ones row)
            g3 = consts.tile([NP, JP], bf16)
            nc.vector.memset(g3, 0.0)       # pad col 19 stays 0
            e4 = consts.tile([NP, PQP], bf16)
            nc.vector.memset(e4, 0.0)       # pad col 27 stays 0
            t1 = atp.tile([NP, PQ * JP], bf16, tag="t1")
            t2 = atp.tile([NP, J * PQP], bf16, tag="t2")
            dots = atp.tile([NP, PQ], f32, tag="dots")
            e4n = atp.tile([NP, PQP], bf16, tag="e4n")
            s4 = atp.tile([NP, J], f32, tag="s4")
            rs = atp.tile([NP, 1], f32, tag="rs")
            recip = atp.tile([NB, 1], f32, tag="recip")
            g_sb = atp.tile([NB, J], f32, tag="gsb")

            # residual stream: x^T as (128, 4*32), f32 master + bf16 copy
            xT_f = apool.tile([128, 4 * NB], f32, tag="xf")
            nc.vector.memset(xT_f, 0.0)
            xT_b = apool.tile([128, 4 * NB], bf16, tag="xb")
            nc.vector.memset(xT_b, 0.0)

            def xsl(t, kt):
                return t[:, kt * NB : (kt + 1) * NB]

            for l in range(L):
                wb = wpool.tile([128, 3 * 2048], fp8, tag="wb")
                nc.sync.dma_start(out=wb, in_=d_wbig[l, :, :])

                def wtile(mat, o, kt):
                    c = ((mat * 4 + o) * 4 + kt) * 128
                    return wb[:, c : c + 128]

                qk_l = qkall[:, l * 4 * J : (l + 1) * 4 * J]
                abc = bcast_free(b01[:, l * 9 + 8 : l * 9 + 9], NB)       # alpha bcast
                abc4 = bcast_free(b01[:, l * 9 + 8 : l * 9 + 9], 4 * NB)

                # ---- attention: g = x @ qk_w + qk_b  (32,19) ----
                g_ps = pt.tile([NB, J], f32, tag="sp")
                if l > 0:
                    for kt in range(4):
                        nc.tensor.matmul(
                            g_ps, xsl(xT_b, kt), qk_l[:, kt * J : (kt + 1) * J],
                            start=(kt == 0), stop=False,
                        )
                nc.tensor.matmul(
                    g_ps, ones1, qb[:, l * J : (l + 1) * J],
                    start=(l == 0), stop=True,
                )
                nc.scalar.activation(g_sb, g_ps, Act.Copy)
                # replicate per-board g to 96 partitions: g3 = e2t.T @ g
                g3_ps = pt.tile([NP, J], f32, tag="sp")
                nc.tensor.matmul(g3_ps, e2t, g_sb, start=True, stop=True)
                nc.scalar.activation(g3[:, 0:J], g3_ps, Act.Copy)

                # fin x-part runs on PE while attention math runs on DVE
                ft = pm.tile([128, 4 * NB], f32, tag="mm")
                if l > 0:
                    for o in range(4):
                        for kt in range(4):
                            nc.tensor.matmul(
                                xsl(ft, o), wtile(0, o, kt), xsl(xT_b, kt),
                                start=(kt == 0), stop=False,
                            )

                # dots[b,p] = bfeat . g  -> (96, 27)
                t1_3 = t1[:, :].rearrange("p (a b) -> p a b", b=JP)
                nc.vector.tensor_tensor(
                    t1_3, bfpj3, bcast_mid(g3[:, :], PQ), op=AluOp.mult
                )
                nc.vector.tensor_reduce(
                    dots, t1_3, axis=mybir.AxisListType.X, op=AluOp.add
                )
                # e = exp(dots) (|dots| small -> no max-shift), rowsum fused
                nc.scalar.activation(e4[:, 0:PQ], dots, Act.Exp, accum_out=rs)
                # rowsum per board: rs_b = e2.T @ rs -> (32,1); recip; replicate
                rsb_ps = pt.tile([NB, 1], f32, tag="sp")
                nc.tensor.matmul(rsb_ps, e2, rs, start=True, stop=True)
                nc.vector.reciprocal(recip, rsb_ps)
                r3_ps = pt.tile([NP, 1], f32, tag="sp")
                nc.tensor.matmul(r3_ps, e2t, recip, start=True, stop=True)
                nc.vector.tensor_scalar_mul(e4n, e4, r3_ps[:, :])
                # s[b,j] = sum_p attn[b,p] bfeat[b,p,j] (normalized e)
                t2_3 = t2[:, :].rearrange("p (a b) -> p a b", b=PQP)
                nc.vector.tensor_tensor(
                    t2_3, bfjp3, bcast_mid(e4n[:, :], J), op=AluOp.mult
                )
                nc.vector.tensor_reduce(
                    s4, t2_3, axis=mybir.AxisListType.X, op=AluOp.add
                )
                # group-sum to s^T directly: (19,32) = s4.T @ e2
                sT_ps = pt.tile([J, NB], f32, tag="sp")
                nc.tensor.matmul(sT_ps, s4, e2, start=True, stop=True)
                nc.scalar.activation(sT_buf[0:J, :], sT_ps, Act.Copy)

                # ---- fin tail: += s @ [sfin;cfin] (K=20, bias inside) ----
                for o in range(4):
                    nc.tensor.matmul(
                        xsl(ft, o),
                        sfall[:, l * D + o * 128 : l * D + (o + 1) * 128],
                        sT_buf,
                        start=(l == 0), stop=True,
                    )
                # x += alpha * relu(ft)  (fused relu+scale, then add)
                tv = apool.tile([128, 4 * NB], bf16, tag="tv")
                nc.vector.scalar_tensor_tensor(
                    tv, ft, 0.0, abc4, op0=AluOp.max, op1=AluOp.mult
                )
                nxf = apool.tile([128, 4 * NB], f32, tag="xf")
                nc.vector.tensor_tensor(nxf, xT_f, tv, op=AluOp.add)
                nxb = apool.tile([128, 4 * NB], bf16, tag="xb")
                nc.scalar.activation(nxb, nxf, Act.Copy)
                xT_f, xT_b = nxf, nxb

                # ---- u = relu(x@fc0 + b0) ----
                u_ps = pm.tile([128, 4 * NB], f32, tag="mm")
                for o in range(4):
                    for kt in range(4):
                        nc.tensor.matmul(
                            xsl(u_ps, o), wtile(1, o, kt), xsl(xT_b, kt),
                            start=(kt == 0), stop=(kt == 3),
                        )
                uT = apool.tile([128, 4 * NB], bf16, tag="uT")
                for o in range(4):
                    nc.scalar.activation(
                        xsl(uT, o), xsl(u_ps, o), Act.Relu,
                        bias=b01[:, l * 9 + o : l * 9 + o + 1],
                    )

                # ---- y = u@fc1 + b1 ; x += alpha * y ----
                y_ps = pm.tile([128, 4 * NB], f32, tag="mm")
                for o in range(4):
                    for kt in range(4):
                        nc.tensor.matmul(
                            xsl(y_ps, o), wtile(2, o, kt), xsl(uT, kt),
                            start=(kt == 0), stop=(kt == 3),
                        )
                yv = apool.tile([128, 4 * NB], bf16, tag="tv")
                for o in range(4):
                    nc.vector.scalar_tensor_tensor(
                        xsl(yv, o), xsl(y_ps, o),
                        b01[:, l * 9 + 4 + o : l * 9 + 5 + o],
                        abc, op0=AluOp.add, op1=AluOp.mult,
                    )
                nxf = apool.tile([128, 4 * NB], f32, tag="xf")
                nc.vector.tensor_tensor(nxf, xT_f, yv, op=AluOp.add)
                nxb = apool.tile([128, 4 * NB], bf16, tag="xb")
                nc.scalar.activation(nxb, nxf, Act.Copy)
                xT_f, xT_b = nxf, nxb

            # ---- head: logits = log_softmax((x @ head_w) @ posT) ----
            zT_ps = pt.tile([POSD, NB], f32, tag="sp")
            for kt in range(4):
                nc.tensor.matmul(
                    zT_ps, whead[:, kt * POSD : (kt + 1) * POSD], xsl(xT_b, kt),
                    start=(kt == 0), stop=(kt == 3),
                )
            zT = apool.tile([POSD, NB], bf16, tag="zT")
            nc.scalar.activation(zT, zT_ps, Act.Copy)
            lg_ps = pt.tile([NB, P], f32, tag="sp")
            nc.tensor.matmul(lg_ps, zT, wpost, start=True, stop=True)
            lg = apool.tile([NB, P], f32, tag="lg")
            nc.scalar.activation(lg, lg_ps, Act.Copy)
            mx = apool.tile([NB, 1], f32, tag="mx")
            nc.vector.tensor_reduce(
                mx, lg[:, :], axis=mybir.AxisListType.X, op=AluOp.max
            )
            negmx = apool.tile([NB, 1], f32, tag="nmx")
            nc.vector.tensor_scalar_mul(negmx, mx, -1.0)
            ex = apool.tile([NB, P], f32, tag="ex")
            sume = apool.tile([NB, 1], f32, tag="sume")
            nc.scalar.activation(
                ex, lg, Act.Exp, bias=negmx[:, :], accum_out=sume
            )
            lse = apool.tile([NB, 1], f32, tag="lse")
            nc.scalar.activation(lse, sume, Act.Ln)
            c = apool.tile([NB, 1], f32, tag="c")
            nc.vector.tensor_add(c, mx, lse)
            outf = apool.tile([NB, P], f32, tag="outf")
            nc.vector.tensor_scalar(
                outf, lg[:, :], c[:, :], None, op0=AluOp.subtract
            )
            nc.sync.dma_start(out=d_out[:, :], in_=outf)

    nc.finalize()
    return nc


def kernel(**inputs):
    inp = {k: np.asarray(v, dtype=np.float32) for k, v in inputs.items()}
    pos = _positions()
    bfeat = _prepare(inp["obs"], pos)  # (256, 81, 19)
    qk_w, qk_b, afin, sfin, cfin = _fold(inp)
    alpha = inp["alpha"].astype(np.float32)

    # big fp8 weights, k/o-tiled into stationary lhsT layout
    wbig = np.concatenate(
        [_ktile_lhsT(afin), _ktile_lhsT(inp["fc0_w"]), _ktile_lhsT(inp["fc1_w"])],
        axis=2,
    ).astype(fp8_np)  # (L, 128, 6144)

    qkall = np.ascontiguousarray(
        qk_w.reshape(L, 4, 128, J).transpose(2, 0, 1, 3)
    ).reshape(128, L * 4 * J).astype(bf16_np)

    sfin_aug = np.concatenate([sfin, cfin[:, None, :]], axis=1)  # (L, 20, 512)
    sfall = np.ascontiguousarray(
        sfin_aug.transpose(1, 0, 2)
    ).reshape(JP, L * D).astype(bf16_np)

    b01 = np.zeros((128, L * 9), np.float32)
    for l in range(L):
        b01[:, l * 9 : l * 9 + 4] = inp["fc0_b"][l].reshape(4, 128).T
        b01[:, l * 9 + 4 : l * 9 + 8] = inp["fc1_b"][l].reshape(4, 128).T
        b01[:, l * 9 + 8] = alpha[l]
    qbv = qk_b.reshape(1, L * J).astype(np.float32)
    whead = (
        inp["head_w"].reshape(4, 128, POSD).transpose(1, 0, 2)
        .reshape(128, 4 * POSD).astype(bf16_np)
    )

    # per-core packed constants
    e2 = np.zeros((NP, NB), np.float32)   # group-sum: e2[b*3+g, b] = 1
    for b in range(NB):
        for g in range(P3):
            e2[b * P3 + g, b] = 1.0
    cpk32 = np.zeros((NP, 128), np.float32)
    cpk32[:, 0:NB] = e2
    cpk32[:NB, NB:128] = e2.T

    in_maps = []
    for cc in range(NCORES):
        bf = bfeat[cc * NB : (cc + 1) * NB]          # (32, 81, 19)
        bf3 = bf.reshape(NB, P3, PQ, J)
        cpk16 = np.zeros((NP, C16_END), np.float32)
        bfpj = np.zeros((NP, PQ, JP), np.float32)
        bfpj[:, :, :J] = bf3.reshape(NP, PQ, J)
        cpk16[:, C_BFPJ:C_BFJP] = bfpj.reshape(NP, PQ * JP)
        bfjp = np.zeros((NP, J, PQP), np.float32)
        bfjp[:, :, :PQ] = bf3.transpose(0, 1, 3, 2).reshape(NP, J, PQ)
        cpk16[:, C_BFJP:C_POST] = bfjp.reshape(NP, J * PQP)
        cpk16[:POSD, C_POST:C16_END] = pos.reshape(P, POSD).T
        in_maps.append({
            "cpk16": cpk16.astype(bf16_np),
            "cpk32": cpk32,
            "wbig": wbig, "qkall": qkall, "sfall": sfall,
            "b01": b01, "qb": qbv, "whead": whead,
        })

    nc = _build_nc([float(a) for a in alpha])
    res = run_bass_kernel_spmd(nc, in_maps, core_ids=list(range(NCORES)))
    out = np.concatenate([r["out"] for r in res.results], axis=0)  # (256, 81)
    return out.astype(np.float32)


# revision 9
# speedup vs baseline: 2.3319x; 1.4314x over previous
# Trainium2 Bass kernel for nn_AttnModel_64098091926054.
#
# Strategy: pure data parallel over batch (256 boards -> 32 per core x 8 cores).
# Host-side constant folding (softmax shift-invariance kills the x-dependent
# k-term; q_w folds into qk_w (512x19); kvx_v/Wv fold through fin_w).
#
# v3: latency-oriented. The kernel is serial-dependency-bound, so:
#  - Transposed compute layout: residual is x^T (128, 4x32) bf16 (pure bf16
#    accumulation; validated ~1e-3), weights stationary fp8e4 (FWL), skinny
#    bf16 activations stream. No transposes, no f32 bias matmuls.
#  - Pipelined attention: g_{l+1} = x_mid@qk + u~@(sign(a)*fc1@qk) + const
#    (fc1@qk folded on host), so layer l+1's attention chain starts right
#    after layer l's fc0-relu; fc1 matmuls + residual hide under it.
#  - fc0/fc1 biases enter PSUM via early K=1 rank-1 matmuls (no deps, run
#    during the attention window); |alpha| rides in the fc0-relu scale so
#    the fc1 residual is a single fused TT; fin bias rides in the K=20
#    sfin-augmented matmul.
#  - Attention math on DVE in bf16 (96 partitions = 32 boards x 3 cell
#    groups, inner dims padded even for 2x mode); tensor_reduce is 1x-only
#    so a 2x TT-fold halves each reduce input first. Row-sum via DVE
#    (not ACT accum) so the softmax-normalization matmul chain
#    (rsb/recip/r3) hides under t2/s4; s4 is normalized instead of e.
#  - e2/e2t group-sum matrices in bf16 padded to 128 for fast LDWEIGHTS.

import numpy as np
import ml_dtypes

import concourse.bass as bass
import concourse.bacc as bacc
import concourse.mybir as mybir
import concourse.tile as tile
from concourse.bass_utils import run_bass_kernel_spmd

BS, D, L, B, P, POSD, J = 9, 512, 8, 256, 81, 12, 19
NCORES = 8
NB = B // NCORES          # 32 boards per core
P3, PQ = 3, 27            # 81 cells = 3 partition-groups x 27
NP = NB * P3              # 96 partitions for attention math
JP = J + 1                # 20: j padded to even for DVE 2x mode
PQP = PQ + 1              # 28: pq padded to even
OFFSETS = [(-1, 0), (-1, 1), (0, -1), (0, 0), (0, 1), (-1, -1), (-1, 0)]

f32 = mybir.dt.float32
bf16 = mybir.dt.bfloat16
fp8 = mybir.dt.float8e4
bf16_np = ml_dtypes.bfloat16
fp8_np = ml_dtypes.float8_e4m3

# cpk16 columns (bf16): bfpj (27x20) | bfjp (19x28) | wpost | e2pad | e2tpad
C_BFPJ = 0
C_BFJP = C_BFPJ + PQ * JP          # 540
C_POST = C_BFJP + J * PQP          # 1072
C_E2 = C_POST + P                  # 1153
C_E2T = C_E2 + NB                  # 1185
C16_END = C_E2T + 128              # 1313
AluOp = mybir.AluOpType
Act = mybir.ActivationFunctionType


def _positions():
    lin = np.linspace(0.0, 1.0, BS, dtype=np.float32)
    rs, cs = np.meshgrid(lin, lin, indexing="ij")
    zs = (rs + cs) / 2.0
    xs = np.stack([rs, cs, zs], -1).astype(np.float32)
    feats = []
    for p in [4.0 / (BS - 1), 16.0 / (BS - 1)]:
        a = (2.0 * np.pi * xs / p).astype(np.float32)
        feats.append(np.concatenate([np.cos(a), np.sin(a)], -1).astype(np.float32))
    return np.concatenate(feats, -1)  # (9, 9, 12)


def _prepare(obs, pos):
    single = obs[..., 0] - obs[..., 1]
    aug = np.pad(single, ((0, 0), (1, 1), (1, 1)))
    w = aug.shape[-1]
    outs = [aug[:, 1 + r : w - 1 + r, 1 + c : w - 1 + c] for (r, c) in OFFSETS]
    neigh = np.stack(outs, -1)
    n = obs.shape[0]
    stack = np.concatenate(
        [neigh, np.broadcast_to(pos, (n,) + pos.shape)], -1
    ).astype(np.float32)
    return stack.reshape(n, P, J)  # (B, 81, 19)


def _fold(inp):
    """Host-side constant folding of weights. All f32 numpy, unscaled."""
    scale = np.float32(1.0 / np.sqrt(D))
    Wk = inp["kvb_w"][:, :, :D]                                   # (L,19,512)
    Wv = inp["kvb_w"][:, :, D:]
    kvx_v = inp["kvx_w"][:, :, D:]                                # (L,512,512)
    qk_w = np.einsum("ldh,ljh->ldj", inp["q_w"], Wk) * scale      # (L,512,19)
    qk_b = np.einsum("lh,ljh->lj", inp["q_b"], Wk) * scale        # (L,19)
    afin = np.einsum("lde,leh->ldh", kvx_v, inp["fin_w"])         # (L,512,512)
    sfin = np.einsum("lje,leh->ljh", Wv, inp["fin_w"])            # (L,19,512)
    bias_v = inp["kvx_b"][:, D:] + inp["kvb_b"][:, D:]
    cfin = np.einsum("le,leh->lh", bias_v, inp["fin_w"]) + inp["fin_b"]
    return qk_w, qk_b, afin, sfin, cfin


def _ktile_lhsT(W):
    """(L,512,512) -> (L,128,2048) with col ((o*4+kt)*128+m) = W[l,kt*128+k,o*128+m]."""
    Lx = W.shape[0]
    return np.ascontiguousarray(
        W.reshape(Lx, 4, 128, 4, 128).transpose(0, 2, 3, 1, 4).reshape(Lx, 128, 2048)
    )


def _build_nc(alpha):
    nc = bacc.Bacc("TRN2", target_bir_lowering=False, debug=False)

    d_cpk16 = nc.dram_tensor("cpk16", [128, C16_END], bf16, kind="ExternalInput")
    d_wbig = nc.dram_tensor("wbig", [L, 128, 3 * 2048], fp8, kind="ExternalInput")
    # per layer: qk k-tiles (4*19) then fq k-tiles (4*19)
    d_qkfq = nc.dram_tensor("qkfq", [128, L * 8 * J], bf16, kind="ExternalInput")
    d_sfall = nc.dram_tensor("sfall", [JP, L * D], bf16, kind="ExternalInput")
    d_bias = nc.dram_tensor("bias", [1, L * 2 * D], bf16, kind="ExternalInput")
    d_gconst = nc.dram_tensor("gconst", [1, L * J], f32, kind="ExternalInput")
    d_whead = nc.dram_tensor("whead", [128, 4 * POSD], bf16, kind="ExternalInput")
    d_out = nc.dram_tensor("out", [NB, P], f32, kind="ExternalOutput")

    def bcast_mid(ap2d, n):
        # (p, k) AP -> (p, n, k) with step-0 broadcast in the middle
        return bass.AP(
            tensor=ap2d.tensor, offset=ap2d.offset,
            ap=[ap2d.ap[0], [0, n], ap2d.ap[1]],
        )

    def bcast_free(ap_col, n):
        # (p, 1) AP -> (p, n) with step-0 broadcast along free dim
        return bass.AP(
            tensor=ap_col.tensor, offset=ap_col.offset,
            ap=[ap_col.ap[0], [0, n]],
        )

    with tile.TileContext(nc) as tc:
        with (
            tc.tile_pool(name="consts", bufs=1) as consts,
            tc.tile_pool(name="wpool", bufs=8) as wpool,
            tc.tile_pool(name="ap", bufs=3) as apool,
            tc.tile_pool(name="attn", bufs=1) as atp,
            tc.tile_pool(name="pm", bufs=4, space="PSUM") as pm,
            tc.tile_pool(name="pt", bufs=2, space="PSUM") as pt,
        ):
            # ---- constants (6 DMAs) ----
            cpk16 = consts.tile([128, C16_END], bf16)
            nc.sync.dma_start(out=cpk16, in_=d_cpk16[:, :])
            qkfq = consts.tile([128, L * 8 * J], bf16)
            nc.sync.dma_start(out=qkfq, in_=d_qkfq[:, :])
            sfall = consts.tile([JP, L * D], bf16)
            nc.sync.dma_start(out=sfall, in_=d_sfall[:, :])
            biasall = consts.tile([1, L * 2 * D], bf16)
            nc.sync.dma_start(out=biasall, in_=d_bias[:, :])
            gconst = consts.tile([1, L * J], f32)
            nc.sync.dma_start(out=gconst, in_=d_gconst[:, :])
            whead = consts.tile([128, 4 * POSD], bf16)
            nc.sync.dma_start(out=whead, in_=d_whead[:, :])

            bfpj3 = cpk16[:NP, C_BFPJ:C_BFJP].rearrange("p (a b) -> p a b", b=JP)
            bfjp3 = cpk16[:NP, C_BFJP:C_POST].rearrange("p (a b) -> p a b", b=PQP)
            wpost = cpk16[:POSD, C_POST:C_E2]     # (12, 81) bf16
            e2p = cpk16[:128, C_E2:C_E2T]         # (128, 32) bf16, rows 96+ = 0
            e2tp = cpk16[:NB, C_E2T:C16_END]      # (32, 128) bf16, cols 96+ = 0

            ones1 = consts.tile([1, NB], f32)
            nc.vector.memset(ones1, 1.0)
            ones_bf = consts.tile([1, NB], bf16)
            nc.vector.memset(ones_bf, 1.0)
            # persistent attention buffers (serial chain -> single-buffered)
            sT_buf = consts.tile([JP, NB], bf16)
            nc.vector.memset(sT_buf, 1.0)   # row 19 stays 1.0 (cfin ones row)
            g3 = consts.tile([NP, JP], bf16)
            nc.vector.memset(g3, 0.0)       # pad col 19 stays 0
            e4 = consts.tile([NP, PQP], bf16)
            nc.vector.memset(e4, 0.0)       # pad col 27 stays 0
            s4 = consts.tile([128, J], bf16)
            nc.vector.memset(s4, 0.0)       # rows 96..127 stay 0
            rsE = consts.tile([128, 1], bf16)
            nc.vector.memset(rsE, 0.0)      # rows 96..127 stay 0
            g_sb = atp.tile([NB, J], bf16, tag="gsb")
            t1 = atp.tile([NP, PQ * JP], bf16, tag="t1")
            t1h = atp.tile([NP, PQ * (JP // 2)], bf16, tag="t1h")
            t2 = atp.tile([NP, J * PQP], bf16, tag="t2")
            t2h = atp.tile([NP, J * (PQP // 2)], bf16, tag="t2h")
            dots = atp.tile([NP, PQ], f32, tag="dots")
            s4n = atp.tile([128, J], bf16, tag="s4n")
            recip = atp.tile([NB, 1], bf16, tag="recip")

            # residual stream: x^T as (128, 4*32), pure bf16
            xT_b = apool.tile([128, 4 * NB], bf16, tag="xb")
            nc.vector.memset(xT_b, 0.0)
            xT_mid = xT_b
            uT = None

            def xsl(t, kt):
                return t[:, kt * NB : (kt + 1) * NB]

            t1_3 = t1[:, :].rearrange("p (a b) -> p a b", b=JP)
            t1_2 = t1[:, :].rearrange("p (a b) -> p a b", b=JP // 2)
            t1h_3 = t1h[:, :].rearrange("p (a b) -> p a b", b=JP // 2)
            t2_3 = t2[:, :].rearrange("p (a b) -> p a b", b=PQP)
            t2_2 = t2[:, :].rearrange("p (a b) -> p a b", b=PQP // 2)
            t2h_3 = t2h[:, :].rearrange("p (a b) -> p a b", b=PQP // 2)

            prev = None  # (wb, uT, y_ps, sign) of layer l-1 pending fc1
            for l in range(L):
                wb = wpool.tile([128, 3 * 2048], fp8, tag="wb")
                nc.sync.dma_start(out=wb, in_=d_wbig[l, :, :])

                def wtile(mat, o, kt, wb=wb):
                    c = ((mat * 4 + o) * 4 + kt) * 128
                    return wb[:, c : c + 128]

                qk_l = qkfq[:, l * 8 * J : l * 8 * J + 4 * J]
                fq_l = qkfq[:, l * 8 * J + 4 * J : (l + 1) * 8 * J]
                aab = abs(alpha[l])

                # ---- g via pipelined fold: x_mid@qk + u~@fq + gconst ----
                g_ps = pt.tile([NB, J], f32, tag="sp")
                if l > 0:
                    wb_p, uT_p, y_p, sgn_p = prev
                    for kt in range(4):
                        nc.tensor.matmul(
                            g_ps, xsl(xT_mid, kt), qk_l[:, kt * J : (kt + 1) * J],
                            start=(kt == 0), stop=False,
                        )
                    for kt in range(4):
                        nc.tensor.matmul(
                            g_ps, xsl(uT_p, kt), fq_l[:, kt * J : (kt + 1) * J],
                            start=False, stop=False,
                        )
                nc.tensor.matmul(
                    g_ps, ones1, gconst[:, l * J : (l + 1) * J],
                    start=(l == 0), stop=True,
                )
                nc.vector.tensor_copy(g_sb, g_ps)
                g3_ps = pt.tile([128, J], f32, tag="sp")
                nc.tensor.matmul(g3_ps, e2tp, g_sb, start=True, stop=True)
                nc.vector.tensor_copy(g3[:, 0:J], g3_ps[0:NP, :])

                # ---- deferred fc1 of layer l-1 (hides under attention) ----
                if l > 0:
                    for o in range(4):
                        for kt in range(4):
                            nc.tensor.matmul(
                                xsl(y_p, o),
                                bass.AP(
                                    tensor=wb_p.tensor, offset=wb_p.offset,
                                    ap=wb_p.ap,
                                )[:, ((2 * 4 + o) * 4 + kt) * 128
                                  : ((2 * 4 + o) * 4 + kt) * 128 + 128],
                                xsl(uT_p, kt),
                                start=False, stop=(kt == 3),
                            )
                    nxb = apool.tile([128, 4 * NB], bf16, tag="xb")
                    nc.vector.tensor_tensor(
                        nxb, xT_mid, y_p,
                        op=AluOp.add if sgn_p >= 0 else AluOp.subtract,
                    )
                    xT_b = nxb

                # fin x-part on PE (hides under attention DVE)
                ft = pm.tile([128, 4 * NB], f32, tag="mm")
                if l > 0:
                    for o in range(4):
                        for kt in range(4):
                            nc.tensor.matmul(
                                xsl(ft, o), wtile(0, o, kt), xsl(xT_b, kt),
                                start=(kt == 0), stop=False,
                            )

                # early bias rank-1s for fc0/fc1 (no deps; run in PE slack)
                u_ps = pm.tile([128, 4 * NB], f32, tag="mm")
                y_ps = pm.tile([128, 4 * NB], f32, tag="mm")
                for o in range(4):
                    nc.tensor.matmul(
                        xsl(u_ps, o),
                        biasall[:, l * 2 * D + o * 128 : l * 2 * D + (o + 1) * 128],
                        ones_bf, start=True, stop=False,
                    )
                    nc.tensor.matmul(
                        xsl(y_ps, o),
                        biasall[:, l * 2 * D + D + o * 128 : l * 2 * D + D + (o + 1) * 128],
                        ones_bf, start=True, stop=False,
                    )

                # ---- dots = bfeat . g -> (96, 27); fold then 1x reduce ----
                nc.vector.tensor_tensor(
                    t1_3, bfpj3, bcast_mid(g3[:, :], PQ), op=AluOp.mult
                )
                nc.vector.tensor_tensor(
                    t1h_3, t1_3[:, :, 0 : JP // 2], t1_3[:, :, JP // 2 : JP],
                    op=AluOp.add,
                )
                nc.vector.tensor_reduce(
                    dots, t1h_3, axis=mybir.AxisListType.X, op=AluOp.add
                )
                nc.scalar.activation(e4[:, 0:PQ], dots, Act.Exp)
                with nc.allow_low_precision(reason="softmax rowsum in bf16"):
                    nc.vector.tensor_reduce(
                        rsE[0:NP, :], e4, axis=mybir.AxisListType.X, op=AluOp.add
                    )
                # rowsum per board + recip + replicate (PE) — hides under t2/s4
                rsb_ps = pt.tile([NB, 1], f32, tag="sp")
                nc.tensor.matmul(rsb_ps, e2p, rsE, start=True, stop=True)

                # ---- s4 = sum_p e[b,p] bfeat[b,p,j] (unnormalized) ----
                nc.vector.tensor_tensor(
                    t2_3, bfjp3, bcast_mid(e4[:, :], J), op=AluOp.mult
                )
                nc.vector.tensor_tensor(
                    t2h_3, t2_3[:, :, 0 : PQP // 2], t2_3[:, :, PQP // 2 : PQP],
                    op=AluOp.add,
                )
                with nc.allow_low_precision(reason="attention s in bf16"):
                    nc.vector.tensor_reduce(
                        s4[0:NP, :], t2h_3, axis=mybir.AxisListType.X, op=AluOp.add
                    )
                with nc.allow_low_precision(reason="softmax recip in bf16"):
                    nc.vector.reciprocal(recip, rsb_ps)
                r3_ps = pt.tile([128, 1], f32, tag="sp")
                nc.tensor.matmul(r3_ps, e2tp, recip, start=True, stop=True)
                nc.vector.tensor_scalar_mul(s4n, s4, r3_ps[:, :])
                # group-sum to s^T directly: (19,32) = s4n.T @ e2
                sT_ps = pt.tile([J, NB], f32, tag="sp")
                nc.tensor.matmul(sT_ps, s4n, e2p, start=True, stop=True)
                nc.vector.tensor_copy(sT_buf[0:J, :], sT_ps)

                # ---- fin tail: += s @ [sfin;cfin] (K=20, bias inside) ----
                for o in range(4):
                    nc.tensor.matmul(
                        xsl(ft, o),
                        sfall[:, l * D + o * 128 : l * D + (o + 1) * 128],
                        sT_buf,
                        start=(l == 0), stop=True,
                    )
                # x_mid = x + alpha*relu(ft)
                tv = apool.tile([128, 4 * NB], bf16, tag="tv")
                nc.vector.tensor_scalar(
                    tv, ft, 0.0, float(alpha[l]), op0=AluOp.max, op1=AluOp.mult
                )
                nmid = apool.tile([128, 4 * NB], bf16, tag="xb")
                nc.vector.tensor_tensor(nmid, xT_b, tv, op=AluOp.add)
                xT_mid = nmid

                # ---- u~ = |alpha|*relu(x_mid@fc0 + b0) ----
                for o in range(4):
                    for kt in range(4):
                        nc.tensor.matmul(
                            xsl(u_ps, o), wtile(1, o, kt), xsl(xT_mid, kt),
                            start=False, stop=(kt == 3),
                        )
                nuT = apool.tile([128, 4 * NB], bf16, tag="uT")
                for o in range(2):
                    nc.scalar.activation(
                        xsl(nuT, o), xsl(u_ps, o), Act.Relu, scale=aab
                    )
                for o in range(2, 4):
                    nc.vector.tensor_scalar(
                        xsl(nuT, o), xsl(u_ps, o), 0.0, aab,
                        op0=AluOp.max, op1=AluOp.mult,
                    )
                uT = nuT
                # fc1 matmuls deferred to next layer (after its g matmuls)
                prev = (wb, uT, y_ps, 1.0 if alpha[l] >= 0 else -1.0)

            # flush last layer's fc1 + residual
            wb_p, uT_p, y_p, sgn_p = prev
            for o in range(4):
                for kt in range(4):
                    nc.tensor.matmul(
                        xsl(y_p, o), wb_p[:, ((2 * 4 + o) * 4 + kt) * 128
                                          : ((2 * 4 + o) * 4 + kt) * 128 + 128],
                        xsl(uT_p, kt),
                        start=False, stop=(kt == 3),
                    )
            nxb = apool.tile([128, 4 * NB], bf16, tag="xb")
            nc.vector.tensor_tensor(
                nxb, xT_mid, y_p,
                op=AluOp.add if sgn_p >= 0 else AluOp.subtract,
            )
            xT_b = nxb

            # ---- head: logits = log_softmax((x @ head_w) @ posT) ----
            zT_ps = pt.tile([POSD, NB], f32, tag="sp")
            for kt in range(4):
                nc.tensor.matmul(
                    zT_ps, whead[:, kt * POSD : (kt + 1) * POSD], xsl(xT_b, kt),
                    start=(kt == 0), stop=(kt == 3),
                )
            zT = apool.tile([POSD, NB], bf16, tag="zT")
            nc.vector.tensor_copy(zT, zT_ps)
            lg_ps = pt.tile([NB, P], f32, tag="sp")
            nc.tensor.matmul(lg_ps, zT, wpost, start=True, stop=True)
            lg = apool.tile([NB, P], f32, tag="lg")
            nc.scalar.activation(lg, lg_ps, Act.Copy)
            mx = apool.tile([NB, 1], f32, tag="mx")
            nc.vector.tensor_reduce(
                mx, lg[:, :], axis=mybir.AxisListType.X, op=AluOp.max
            )
            negmx = apool.tile([NB, 1], f32, tag="nmx")
            nc.vector.tensor_scalar_mul(negmx, mx, -1.0)
            ex = apool.tile([NB, P], f32, tag="ex")
            sume = apool.tile([NB, 1], f32, tag="sume")
            nc.scalar.activation(
                ex, lg, Act.Exp, bias=negmx[:, :], accum_out=sume
            )
            lse = apool.tile([NB, 1], f32, tag="lse")
            nc.scalar.activation(lse, sume, Act.Ln)
            c = apool.tile([NB, 1], f32, tag="c")
            nc.vector.tensor_add(c, mx, lse)
            outf = apool.tile([NB, P], f32, tag="outf")
            nc.vector.tensor_scalar(
                outf, lg[:, :], c[:, :], None, op0=AluOp.subtract
            )
            nc.sync.dma_start(out=d_out[:, :], in_=outf)

    nc.finalize()
    return nc


def kernel(**inputs):
    inp = {k: np.asarray(v, dtype=np.float32) for k, v in inputs.items()}
    pos = _positions()
    bfeat = _prepare(inp["obs"], pos)  # (256, 81, 19)
    qk_w, qk_b, afin, sfin, cfin = _fold(inp)
    alpha = inp["alpha"].astype(np.float32)

    # big fp8 weights, k/o-tiled into stationary lhsT layout
    wbig = np.concatenate(
        [_ktile_lhsT(afin), _ktile_lhsT(inp["fc0_w"]), _ktile_lhsT(inp["fc1_w"])],
        axis=2,
    ).astype(fp8_np)  # (L, 128, 6144)

    # g pipelining folds: fq_l = sign(a_{l-1}) * fc1_{l-1} @ qk_l
    fq = np.zeros((L, D, J), np.float32)
    gconst = qk_b.copy()
    for l in range(1, L):
        sgn = 1.0 if alpha[l - 1] >= 0 else -1.0
        fq[l] = sgn * (inp["fc1_w"][l - 1] @ qk_w[l])
        gconst[l] = qk_b[l] + alpha[l - 1] * (inp["fc1_b"][l - 1] @ qk_w[l])
    qkfq = np.zeros((128, L * 8 * J), np.float32)
    for l in range(L):
        qkfq[:, l * 8 * J : l * 8 * J + 4 * J] = (
            qk_w[l].reshape(4, 128, J).transpose(1, 0, 2).reshape(128, 4 * J)
        )
        qkfq[:, l * 8 * J + 4 * J : (l + 1) * 8 * J] = (
            fq[l].reshape(4, 128, J).transpose(1, 0, 2).reshape(128, 4 * J)
        )

    sfin_aug = np.concatenate([sfin, cfin[:, None, :]], axis=1)  # (L, 20, 512)
    sfall = np.ascontiguousarray(
        sfin_aug.transpose(1, 0, 2)
    ).reshape(JP, L * D).astype(bf16_np)

    # rank-1 bias rows: |a|*b0 then a*b1 per layer
    biasall = np.zeros((1, L * 2 * D), np.float32)
    for l in range(L):
        biasall[0, l * 2 * D : l * 2 * D + D] = abs(alpha[l]) * inp["fc0_b"][l]
        biasall[0, l * 2 * D + D : (l + 1) * 2 * D] = (
            (1.0 if alpha[l] >= 0 else -1.0) * alpha[l] * inp["fc1_b"][l]
        )
    gconst_v = gconst.reshape(1, L * J).astype(np.float32)
    whead = (
        inp["head_w"].reshape(4, 128, POSD).transpose(1, 0, 2)
        .reshape(128, 4 * POSD).astype(bf16_np)
    )

    # per-core packed constants
    e2 = np.zeros((NP, NB), np.float32)   # group-sum: e2[b*3+g, b] = 1
    for b in range(NB):
        for g in range(P3):
            e2[b * P3 + g, b] = 1.0

    in_maps = []
    for cc in range(NCORES):
        bf = bfeat[cc * NB : (cc + 1) * NB]          # (32, 81, 19)
        bf3 = bf.reshape(NB, P3, PQ, J)
        cpk16 = np.zeros((128, C16_END), np.float32)
        bfpj = np.zeros((NP, PQ, JP), np.float32)
        bfpj[:, :, :J] = bf3.reshape(NP, PQ, J)
        cpk16[:NP, C_BFPJ:C_BFJP] = bfpj.reshape(NP, PQ * JP)
        bfjp = np.zeros((NP, J, PQP), np.float32)
        bfjp[:, :, :PQ] = bf3.transpose(0, 1, 3, 2).reshape(NP, J, PQ)
        cpk16[:NP, C_BFJP:C_POST] = bfjp.reshape(NP, J * PQP)
        cpk16[:POSD, C_POST:C_E2] = pos.reshape(P, POSD).T
        cpk16[:NP, C_E2:C_E2T] = e2
        cpk16[:NB, C_E2T : C_E2T + NP] = e2.T
        in_maps.append({
            "cpk16": cpk16.astype(bf16_np),
            "wbig": wbig, "qkfq": qkfq.astype(bf16_np), "sfall": sfall,
            "bias": biasall.astype(bf16_np), "gconst": gconst_v, "whead": whead,
        })

    nc = _build_nc([float(a) for a in alpha])
    res = run_bass_kernel_spmd(nc, in_maps, core_ids=list(range(NCORES)))
    out = np.concatenate([r["out"] for r in res.results], axis=0)  # (256, 81)
    return out.astype(np.float32)


# revision 11
# speedup vs baseline: 2.5691x; 1.1017x over previous
# Trainium2 Bass kernel for nn_AttnModel_64098091926054.
#
# Strategy: pure data parallel over batch (256 boards -> 32 per core x 8 cores).
# Host-side constant folding (softmax shift-invariance kills the x-dependent
# k-term; q_w folds into qk_w (512x19); kvx_v/Wv fold through fin_w).
#
# v4: latency-oriented. The kernel is one serial dependency loop per layer:
#   t1 -> dots -> exp -> t2 -> s4 -> s4n -> sT -> sfin -> tv -> x_mid -> fc0
#   -> relu -> (u@fq) -> g -> t1' ...
# so every link is minimized:
#  - Transposed compute: residual x^T (128, 4x32) pure bf16; weights are the
#    stationary operand (fp8e4), skinny bf16 activations stream (N=32).
#  - Pipelined attention: g_{l+1} = x_mid@qk + u~@(sign(a)*fc1@qk) + const
#    (fc1@qk folded on host) => fc1 matmuls + residual hide under attention.
#  - Group-major 4-way cell split: 81 cells -> 4 groups x 21 (3 pads), so
#    all 128 partitions work and DVE free sizes shrink 27%. Pad cells carry
#    a -40 in the padded j-column against a constant 1.0 in g3's pad column,
#    so exp gives them zero weight with zero extra instructions.
#  - g3 (128, 19) comes from ONE matmul set with a step-0-broadcast AP on
#    the stationary operand (x columns replicated 4x) - no replicate matmul,
#    no g_sb copy.
#  - fc0/fc1 biases enter PSUM via early K=1 rank-1 matmuls; |alpha| rides
#    in a SINGLE (128,128) relu (scale imm); fin bias rides in the K=20
#    sfin matmul; fc1 residual is a single TT add/sub.
#  - softmax normalization (rs/recip/r3) runs on accum_out + PE and hides
#    under t2/s4; s4 is normalized instead of e.

import numpy as np
import ml_dtypes

import concourse.bass as bass
import concourse.bacc as bacc
import concourse.mybir as mybir
import concourse.tile as tile
from concourse.bass_utils import run_bass_kernel_spmd

BS, D, L, B, P, POSD, J = 9, 512, 8, 256, 81, 12, 19
NCORES = 8
NB = B // NCORES          # 32 boards per core
GG, PQ = 4, 21            # 84 = 4 groups x 21 cells (3 pads)
NP = 128                  # partitions: p = gg*32 + b  (group-major)
JP = J + 1                # 20: j padded (pad col doubles as -40 pad-cell lane)
PQP = PQ + 1              # 22: pq padded even
NEGBIG = -40.0
OFFSETS = [(-1, 0), (-1, 1), (0, -1), (0, 0), (0, 1), (-1, -1), (-1, 0)]

f32 = mybir.dt.float32
bf16 = mybir.dt.bfloat16
fp8 = mybir.dt.float8e4
bf16_np = ml_dtypes.bfloat16
fp8_np = ml_dtypes.float8_e4m3

# cpk16 columns (bf16): bfpj (21x20) | bfjp (19x22) | wpost | e2p | e2tp
C_BFPJ = 0
C_BFJP = C_BFPJ + PQ * JP          # 420
C_POST = C_BFJP + J * PQP          # 838
C_E2 = C_POST + P                  # 919
C_E2T = C_E2 + NB                  # 951
C16_END = C_E2T + 128              # 1079
AluOp = mybir.AluOpType
Act = mybir.ActivationFunctionType


def _positions():
    lin = np.linspace(0.0, 1.0, BS, dtype=np.float32)
    rs, cs = np.meshgrid(lin, lin, indexing="ij")
    zs = (rs + cs) / 2.0
    xs = np.stack([rs, cs, zs], -1).astype(np.float32)
    feats = []
    for p in [4.0 / (BS - 1), 16.0 / (BS - 1)]:
        a = (2.0 * np.pi * xs / p).astype(np.float32)
        feats.append(np.concatenate([np.cos(a), np.sin(a)], -1).astype(np.float32))
    return np.concatenate(feats, -1)  # (9, 9, 12)


def _prepare(obs, pos):
    single = obs[..., 0] - obs[..., 1]
    aug = np.pad(single, ((0, 0), (1, 1), (1, 1)))
    w = aug.shape[-1]
    outs = [aug[:, 1 + r : w - 1 + r, 1 + c : w - 1 + c] for (r, c) in OFFSETS]
    neigh = np.stack(outs, -1)
    n = obs.shape[0]
    stack = np.concatenate(
        [neigh, np.broadcast_to(pos, (n,) + pos.shape)], -1
    ).astype(np.float32)
    return stack.reshape(n, P, J)  # (B, 81, 19)


def _fold(inp):
    """Host-side constant folding of weights. All f32 numpy, unscaled."""
    scale = np.float32(1.0 / np.sqrt(D))
    Wk = inp["kvb_w"][:, :, :D]                                   # (L,19,512)
    Wv = inp["kvb_w"][:, :, D:]
    kvx_v = inp["kvx_w"][:, :, D:]                                # (L,512,512)
    qk_w = np.einsum("ldh,ljh->ldj", inp["q_w"], Wk) * scale      # (L,512,19)
    qk_b = np.einsum("lh,ljh->lj", inp["q_b"], Wk) * scale        # (L,19)
    afin = np.einsum("lde,leh->ldh", kvx_v, inp["fin_w"])         # (L,512,512)
    sfin = np.einsum("lje,leh->ljh", Wv, inp["fin_w"])            # (L,19,512)
    bias_v = inp["kvx_b"][:, D:] + inp["kvb_b"][:, D:]
    cfin = np.einsum("le,leh->lh", bias_v, inp["fin_w"]) + inp["fin_b"]
    return qk_w, qk_b, afin, sfin, cfin


def _ktile_lhsT(W):
    """(L,512,512) -> (L,128,2048) with col ((o*4+kt)*128+m) = W[l,kt*128+k,o*128+m]."""
    Lx = W.shape[0]
    return np.ascontiguousarray(
        W.reshape(Lx, 4, 128, 4, 128).transpose(0, 2, 3, 1, 4).reshape(Lx, 128, 2048)
    )


def _build_nc(alpha):
    nc = bacc.Bacc("TRN2", target_bir_lowering=False, debug=False)

    d_cpk16 = nc.dram_tensor("cpk16", [128, C16_END], bf16, kind="ExternalInput")
    d_e2pf = nc.dram_tensor("e2pf", [128, NB], f32, kind="ExternalInput")
    d_wbig = nc.dram_tensor("wbig", [L, 128, 3 * 2048], fp8, kind="ExternalInput")
    # per layer: qk k-tiles (4*19) then fq k-tiles (4*19)
    d_qkfq = nc.dram_tensor("qkfq", [128, L * 8 * J], bf16, kind="ExternalInput")
    d_sfall = nc.dram_tensor("sfall", [JP, L * D], bf16, kind="ExternalInput")
    d_bias = nc.dram_tensor("bias", [1, L * 2 * D], bf16, kind="ExternalInput")
    d_gconst = nc.dram_tensor("gconst", [1, L * J], f32, kind="ExternalInput")
    d_whead = nc.dram_tensor("whead", [128, 4 * POSD], bf16, kind="ExternalInput")
    d_out = nc.dram_tensor("out", [NB, P], f32, kind="ExternalOutput")

    def bcast_mid(ap2d, n):
        # (p, k) AP -> (p, n, k) with step-0 broadcast in the middle
        return bass.AP(
            tensor=ap2d.tensor, offset=ap2d.offset,
            ap=[ap2d.ap[0], [0, n], ap2d.ap[1]],
        )

    with tile.TileContext(nc) as tc:
        with (
            tc.tile_pool(name="consts", bufs=1) as consts,
            tc.tile_pool(name="wpool", bufs=8) as wpool,
            tc.tile_pool(name="ap", bufs=3) as apool,
            tc.tile_pool(name="attn", bufs=1) as atp,
            tc.tile_pool(name="pm", bufs=4, space="PSUM") as pm,
            tc.tile_pool(name="pt", bufs=2, space="PSUM") as pt,
        ):
            # ---- constants (7 DMAs) ----
            cpk16 = consts.tile([128, C16_END], bf16)
            nc.sync.dma_start(out=cpk16, in_=d_cpk16[:, :])
            e2pf = consts.tile([128, NB], f32)
            nc.sync.dma_start(out=e2pf, in_=d_e2pf[:, :])
            qkfq = consts.tile([128, L * 8 * J], bf16)
            nc.sync.dma_start(out=qkfq, in_=d_qkfq[:, :])
            sfall = consts.tile([JP, L * D], bf16)
            nc.sync.dma_start(out=sfall, in_=d_sfall[:, :])
            biasall = consts.tile([1, L * 2 * D], bf16)
            nc.sync.dma_start(out=biasall, in_=d_bias[:, :])
            gconst = consts.tile([1, L * J], f32)
            nc.sync.dma_start(out=gconst, in_=d_gconst[:, :])
            whead = consts.tile([128, 4 * POSD], bf16)
            nc.sync.dma_start(out=whead, in_=d_whead[:, :])

            bfpj4 = cpk16[:NP, C_BFPJ:C_BFJP].rearrange("p (a b) -> p a b", b=JP)
            bfjp4 = cpk16[:NP, C_BFJP:C_POST].rearrange("p (a b) -> p a b", b=PQP)
            wpost = cpk16[:POSD, C_POST:C_E2]     # (12, 81) bf16
            e2p = cpk16[:128, C_E2:C_E2T]         # (128, 32) bf16 group-major
            e2tp = cpk16[:NB, C_E2T:C16_END]      # (32, 128) bf16

            ones1 = consts.tile([1, NB], f32)
            nc.vector.memset(ones1, 1.0)
            ones_bf = consts.tile([1, NB], bf16)
            nc.vector.memset(ones_bf, 1.0)
            # persistent attention buffers (serial chain -> single-buffered)
            sT_buf = consts.tile([JP, NB], bf16)
            nc.vector.memset(sT_buf, 1.0)   # row 19 stays 1.0 (cfin ones row)
            g3 = consts.tile([NP, JP], bf16)
            nc.vector.memset(g3, 1.0)       # pad col 19 stays 1.0 (-40 lane)
            e4 = consts.tile([NP, PQP], bf16)
            nc.vector.memset(e4, 0.0)       # pad col 21 stays 0
            g_sb = atp.tile([NB, J], bf16, tag="gsb")
            s4 = atp.tile([NP, J], bf16, tag="s4")
            rs = atp.tile([NP, 1], f32, tag="rs")
            t1 = atp.tile([NP, PQ * JP], bf16, tag="t1")
            t2 = atp.tile([NP, J * PQP], bf16, tag="t2")
            dots = atp.tile([NP, PQ], f32, tag="dots")
            s4n = atp.tile([NP, J], bf16, tag="s4n")
            recip = atp.tile([NB, 1], bf16, tag="recip")

            # residual stream: x^T as (128, 4*32), pure bf16
            xT_b = apool.tile([128, 4 * NB], bf16, tag="xb")
            nc.vector.memset(xT_b, 0.0)
            xT_mid = xT_b

            def xsl(t, kt):
                return t[:, kt * NB : (kt + 1) * NB]

            t1_3 = t1[:, :].rearrange("p (a b) -> p a b", b=JP)
            t2_3 = t2[:, :].rearrange("p (a b) -> p a b", b=PQP)

            prev = None  # (wb, uT, y_ps, sign) of layer l-1 pending fc1
            for l in range(L):
                wb = wpool.tile([128, 3 * 2048], fp8, tag="wb")
                nc.sync.dma_start(out=wb, in_=d_wbig[l, :, :])

                def wtile(mat, o, kt, wbx=None):
                    wbx = wb if wbx is None else wbx
                    c = ((mat * 4 + o) * 4 + kt) * 128
                    return wbx[:, c : c + 128]

                qk_l = qkfq[:, l * 8 * J : l * 8 * J + 4 * J]
                fq_l = qkfq[:, l * 8 * J + 4 * J : (l + 1) * 8 * J]
                aab = abs(alpha[l])

                # ---- g (32,19) then replicate to 128 group-major rows ----
                g_ps = pt.tile([NB, J], f32, tag="sp")
                nc.tensor.matmul(
                    g_ps, ones1, gconst[:, l * J : (l + 1) * J],
                    start=True, stop=(l == 0),
                )
                if l > 0:
                    wb_p, uT_p, y_p, sgn_p = prev
                    for kt in range(4):
                        nc.tensor.matmul(
                            g_ps, xsl(xT_mid, kt),
                            qk_l[:, kt * J : (kt + 1) * J],
                            start=False, stop=False,
                        )
                    for kt in range(4):
                        nc.tensor.matmul(
                            g_ps, xsl(uT_p, kt),
                            fq_l[:, kt * J : (kt + 1) * J],
                            start=False, stop=(kt == 3),
                        )
                nc.vector.tensor_copy(g_sb, g_ps)
                g3_ps = pt.tile([NP, J], f32, tag="sp")
                nc.tensor.matmul(g3_ps, e2tp, g_sb, start=True, stop=True)
                nc.vector.tensor_copy(g3[:, 0:J], g3_ps)

                # ---- deferred fc1 of layer l-1 (hides under attention) ----
                if l > 0:
                    for o in range(4):
                        for kt in range(4):
                            nc.tensor.matmul(
                                xsl(y_p, o), wtile(2, o, kt, wb_p),
                                xsl(uT_p, kt),
                                start=False, stop=(kt == 3),
                            )

                # psum tiles + early bias rank-1s (no deps; run in PE slack)
                ft = pm.tile([128, 4 * NB], f32, tag="mm")
                u_ps = pm.tile([128, 4 * NB], f32, tag="mm")
                y_ps = pm.tile([128, 4 * NB], f32, tag="mm")
                for o in range(4):
                    nc.tensor.matmul(
                        xsl(u_ps, o),
                        biasall[:, l * 2 * D + o * 128 : l * 2 * D + (o + 1) * 128],
                        ones_bf, start=True, stop=False,
                    )
                    nc.tensor.matmul(
                        xsl(y_ps, o),
                        biasall[:, l * 2 * D + D + o * 128 : l * 2 * D + D + (o + 1) * 128],
                        ones_bf, start=True, stop=False,
                    )

                # ---- dots = bfeat . g -> (128, 21) ----
                nc.vector.tensor_tensor(
                    t1_3, bfpj4, bcast_mid(g3[:, :], PQ), op=AluOp.mult
                )
                nc.vector.tensor_reduce(
                    dots, t1_3, axis=mybir.AxisListType.X, op=AluOp.add
                )
                # fc1 residual of layer l-1 (DVE slot after dots, before t2)
                if l > 0:
                    nxb = apool.tile([128, 4 * NB], bf16, tag="xb")
                    nc.vector.tensor_tensor(
                        nxb, xT_mid, y_p,
                        op=AluOp.add if sgn_p >= 0 else AluOp.subtract,
                    )
                    xT_b = nxb
                    # fin x-part now that x_l is final
                    for o in range(4):
                        for kt in range(4):
                            nc.tensor.matmul(
                                xsl(ft, o), wtile(0, o, kt), xsl(xT_b, kt),
                                start=(kt == 0), stop=False,
                            )
                with nc.allow_low_precision(reason="softmax rowsum"):
                    nc.scalar.activation(
                        e4[:, 0:PQ], dots, Act.Exp, accum_out=rs
                    )
                rsb_ps = pt.tile([NB, 1], f32, tag="sp")
                nc.tensor.matmul(rsb_ps, e2pf, rs, start=True, stop=True)

                # ---- s4 = sum_p e[b,p] bfeat[b,p,j] (unnormalized) ----
                nc.vector.tensor_tensor(
                    t2_3, bfjp4, bcast_mid(e4[:, :], J), op=AluOp.mult
                )
                with nc.allow_low_precision(reason="softmax recip in bf16"):
                    nc.vector.reciprocal(recip, rsb_ps)
                r3_ps = pt.tile([128, 1], f32, tag="sp")
                nc.tensor.matmul(r3_ps, e2tp, recip, start=True, stop=True)
                with nc.allow_low_precision(reason="attention s in bf16"):
                    nc.vector.tensor_reduce(
                        s4, t2_3, axis=mybir.AxisListType.X, op=AluOp.add
                    )
                nc.vector.tensor_scalar_mul(s4n, s4, r3_ps[:, :])
                # group-sum to s^T directly: (19,32) = s4n.T @ e2
                sT_ps = pt.tile([J, NB], f32, tag="sp")
                nc.tensor.matmul(sT_ps, s4n, e2p, start=True, stop=True)
                nc.vector.tensor_copy(sT_buf[0:J, :], sT_ps)

                # ---- fin tail: += s @ [sfin;cfin] (K=20, bias inside) ----
                for o in range(4):
                    nc.tensor.matmul(
                        xsl(ft, o),
                        sfall[:, l * D + o * 128 : l * D + (o + 1) * 128],
                        sT_buf,
                        start=(l == 0), stop=True,
                    )
                # x_mid = x + alpha*relu(ft)
                tv = apool.tile([128, 4 * NB], bf16, tag="tv")
                nc.vector.tensor_scalar(
                    tv, ft, 0.0, float(alpha[l]), op0=AluOp.max, op1=AluOp.mult
                )
                nmid = apool.tile([128, 4 * NB], bf16, tag="xb")
                nc.vector.tensor_tensor(nmid, xT_b, tv, op=AluOp.add)
                xT_mid = nmid

                # ---- u~ = |alpha|*relu(x_mid@fc0 + b0): one ACT op ----
                for o in range(4):
                    for kt in range(4):
                        nc.tensor.matmul(
                            xsl(u_ps, o), wtile(1, o, kt), xsl(xT_mid, kt),
                            start=False, stop=(kt == 3),
                        )
                nuT = apool.tile([128, 4 * NB], bf16, tag="uT")
                nc.scalar.activation(nuT, u_ps, Act.Relu, scale=aab)
                uT = nuT
                # fc1 matmuls deferred to next layer (after its g matmuls)
                prev = (wb, uT, y_ps, 1.0 if alpha[l] >= 0 else -1.0)

            # flush last layer's fc1 + residual
            wb_p, uT_p, y_p, sgn_p = prev
            for o in range(4):
                for kt in range(4):
                    nc.tensor.matmul(
                        xsl(y_p, o), wtile(2, o, kt, wb_p), xsl(uT_p, kt),
                        start=False, stop=(kt == 3),
                    )
            nxb = apool.tile([128, 4 * NB], bf16, tag="xb")
            nc.vector.tensor_tensor(
                nxb, xT_mid, y_p,
                op=AluOp.add if sgn_p >= 0 else AluOp.subtract,
            )
            xT_b = nxb

            # ---- head: logits = log_softmax((x @ head_w) @ posT) ----
            zT_ps = pt.tile([POSD, NB], f32, tag="sp")
            for kt in range(4):
                nc.tensor.matmul(
                    zT_ps, whead[:, kt * POSD : (kt + 1) * POSD], xsl(xT_b, kt),
                    start=(kt == 0), stop=(kt == 3),
                )
            zT = apool.tile([POSD, NB], bf16, tag="zT")
            nc.vector.tensor_copy(zT, zT_ps)
            lg_ps = pt.tile([NB, P], f32, tag="sp")
            nc.tensor.matmul(lg_ps, zT, wpost, start=True, stop=True)
            lg = apool.tile([NB, P], f32, tag="lg")
            nc.scalar.activation(lg, lg_ps, Act.Copy)
            mx = apool.tile([NB, 1], f32, tag="mx")
            nc.vector.tensor_reduce(
                mx, lg[:, :], axis=mybir.AxisListType.X, op=AluOp.max
            )
            negmx = apool.tile([NB, 1], f32, tag="nmx")
            nc.vector.tensor_scalar_mul(negmx, mx, -1.0)
            ex = apool.tile([NB, P], f32, tag="ex")
            sume = apool.tile([NB, 1], f32, tag="sume")
            nc.scalar.activation(
                ex, lg, Act.Exp, bias=negmx[:, :], accum_out=sume
            )
            lse = apool.tile([NB, 1], f32, tag="lse")
            nc.scalar.activation(lse, sume, Act.Ln)
            c = apool.tile([NB, 1], f32, tag="c")
            nc.vector.tensor_add(c, mx, lse)
            outf = apool.tile([NB, P], f32, tag="outf")
            nc.vector.tensor_scalar(
                outf, lg[:, :], c[:, :], None, op0=AluOp.subtract
            )
            nc.sync.dma_start(out=d_out[:, :], in_=outf)

    nc.finalize()
    return nc


def kernel(**inputs):
    inp = {k: np.asarray(v, dtype=np.float32) for k, v in inputs.items()}
    pos = _positions()
    bfeat = _prepare(inp["obs"], pos)  # (256, 81, 19)
    qk_w, qk_b, afin, sfin, cfin = _fold(inp)
    alpha = inp["alpha"].astype(np.float32)

    wbig = np.concatenate(
        [_ktile_lhsT(afin), _ktile_lhsT(inp["fc0_w"]), _ktile_lhsT(inp["fc1_w"])],
        axis=2,
    ).astype(fp8_np)  # (L, 128, 6144)

    # g pipelining folds: fq_l = sign(a_{l-1}) * fc1_{l-1} @ qk_l
    fq = np.zeros((L, D, J), np.float32)
    gconst = qk_b.copy()
    for l in range(1, L):
        sgn = 1.0 if alpha[l - 1] >= 0 else -1.0
        fq[l] = sgn * (inp["fc1_w"][l - 1] @ qk_w[l])
        gconst[l] = qk_b[l] + alpha[l - 1] * (inp["fc1_b"][l - 1] @ qk_w[l])
    qkfq = np.zeros((128, L * 8 * J), np.float32)
    for l in range(L):
        qkfq[:, l * 8 * J : l * 8 * J + 4 * J] = (
            qk_w[l].reshape(4, 128, J).transpose(1, 0, 2).reshape(128, 4 * J)
        )
        qkfq[:, l * 8 * J + 4 * J : (l + 1) * 8 * J] = (
            fq[l].reshape(4, 128, J).transpose(1, 0, 2).reshape(128, 4 * J)
        )

    sfin_aug = np.concatenate([sfin, cfin[:, None, :]], axis=1)  # (L, 20, 512)
    sfall = np.ascontiguousarray(
        sfin_aug.transpose(1, 0, 2)
    ).reshape(JP, L * D).astype(bf16_np)

    biasall = np.zeros((1, L * 2 * D), np.float32)
    for l in range(L):
        biasall[0, l * 2 * D : l * 2 * D + D] = inp["fc0_b"][l]
        biasall[0, l * 2 * D + D : (l + 1) * 2 * D] = (
            (1.0 if alpha[l] >= 0 else -1.0) * alpha[l] * inp["fc1_b"][l]
        )
    gconst_v = gconst.reshape(1, L * J).astype(np.float32)
    whead = (
        inp["head_w"].reshape(4, 128, POSD).transpose(1, 0, 2)
        .reshape(128, 4 * POSD).astype(bf16_np)
    )

    # group-major constants: partition p = gg*32 + b, cell = gg*21 + pq
    e2 = np.zeros((NP, NB), np.float32)
    for gg in range(GG):
        for b in range(NB):
            e2[gg * NB + b, b] = 1.0

    in_maps = []
    for cc in range(NCORES):
        bf = bfeat[cc * NB : (cc + 1) * NB]          # (32, 81, 19)
        # pad cells 81..83 with zeros, cell c -> (gg=c//21, pq=c%21)
        bfp = np.zeros((NB, GG * PQ, J), np.float32)
        bfp[:, :P, :] = bf
        bfg = bfp.reshape(NB, GG, PQ, J).transpose(1, 0, 2, 3)  # (gg,b,pq,j)
        cpk = np.zeros((128, C16_END), np.float32)
        bfpj = np.zeros((GG, NB, PQ, JP), np.float32)
        bfpj[:, :, :, :J] = bfg
        # pad-cell kill switch: -40 in the j-pad lane (g3 pad col is 1.0)
        for c in range(P, GG * PQ):
            bfpj[c // PQ, :, c % PQ, J] = NEGBIG
        cpk[:, C_BFPJ:C_BFJP] = bfpj.reshape(NP, PQ * JP)
        bfjp = np.zeros((GG, NB, J, PQP), np.float32)
        bfjp[:, :, :, :PQ] = bfg.transpose(0, 1, 3, 2)
        cpk[:, C_BFJP:C_POST] = bfjp.reshape(NP, J * PQP)
        cpk[:POSD, C_POST:C_E2] = pos.reshape(P, POSD).T
        cpk[:, C_E2:C_E2T] = e2
        cpk[:NB, C_E2T:C16_END] = e2.T
        in_maps.append({
            "cpk16": cpk.astype(bf16_np), "e2pf": e2,
            "wbig": wbig, "qkfq": qkfq.astype(bf16_np), "sfall": sfall,
            "bias": biasall.astype(bf16_np), "gconst": gconst_v, "whead": whead,
        })

    nc = _build_nc([float(a) for a in alpha])
    res = run_bass_kernel_spmd(nc, in_maps, core_ids=list(range(NCORES)))
    out = np.concatenate([r["out"] for r in res.results], axis=0)  # (256, 81)
    return out.astype(np.float32)


# revision 12
# speedup vs baseline: 2.6573x; 1.0343x over previous
# Trainium2 Bass kernel for nn_AttnModel_64098091926054.
#
# Strategy: pure data parallel over batch (256 boards -> 32 per core x 8 cores).
# Host-side constant folding (softmax shift-invariance kills the x-dependent
# k-term; q_w folds into qk_w (512x19); kvx_v/Wv fold through fin_w).
#
# v4: latency-oriented. The kernel is one serial dependency loop per layer:
#   t1 -> dots -> exp -> t2 -> s4 -> s4n -> sT -> sfin -> tv -> x_mid -> fc0
#   -> relu -> (u@fq) -> g -> t1' ...
# so every link is minimized:
#  - Transposed compute: residual x^T (128, 4x32) pure bf16; weights are the
#    stationary operand (fp8e4), skinny bf16 activations stream (N=32).
#  - Pipelined attention: g_{l+1} = x_mid@qk + u~@(sign(a)*fc1@qk) + const
#    (fc1@qk folded on host) => fc1 matmuls + residual hide under attention.
#  - Group-major 4-way cell split: 81 cells -> 4 groups x 21 (3 pads), so
#    all 128 partitions work and DVE free sizes shrink 27%. Pad cells carry
#    a -40 in the padded j-column against a constant 1.0 in g3's pad column,
#    so exp gives them zero weight with zero extra instructions.
#  - g3 (128, 19) comes from ONE matmul set with a step-0-broadcast AP on
#    the stationary operand (x columns replicated 4x) - no replicate matmul,
#    no g_sb copy.
#  - fc0/fc1 biases enter PSUM via early K=1 rank-1 matmuls; |alpha| rides
#    in a SINGLE (128,128) relu (scale imm); fin bias rides in the K=20
#    sfin matmul; fc1 residual is a single TT add/sub.
#  - softmax normalization (rs/recip/r3) runs on accum_out + PE and hides
#    under t2/s4; s4 is normalized instead of e.

import numpy as np
import ml_dtypes

import concourse.bass as bass
import concourse.bacc as bacc
import concourse.mybir as mybir
import concourse.tile as tile
from concourse.bass_utils import run_bass_kernel_spmd

BS, D, L, B, P, POSD, J = 9, 512, 8, 256, 81, 12, 19
NCORES = 8
NB = B // NCORES          # 32 boards per core
GG, PQ = 4, 21            # 84 = 4 groups x 21 cells (3 pads)
NP = 128                  # partitions: p = gg*32 + b  (group-major)
JP = J + 1                # 20: j padded (pad col doubles as -40 pad-cell lane)
PQP = PQ + 1              # 22: pq padded even
NEGBIG = -40.0
OFFSETS = [(-1, 0), (-1, 1), (0, -1), (0, 0), (0, 1), (-1, -1), (-1, 0)]

f32 = mybir.dt.float32
bf16 = mybir.dt.bfloat16
fp8 = mybir.dt.float8e4
bf16_np = ml_dtypes.bfloat16
fp8_np = ml_dtypes.float8_e4m3

# cpk16 columns (bf16): bfpj (21x20) | bfjp (19x22) | wpost | e2p | e2tp
C_BFPJ = 0
C_BFJP = C_BFPJ + PQ * JP          # 420
C_POST = C_BFJP + J * PQP          # 838
C_E2 = C_POST + P                  # 919
C_E2T = C_E2 + NB                  # 951
C16_END = C_E2T + 128              # 1079
AluOp = mybir.AluOpType
Act = mybir.ActivationFunctionType


def _positions():
    lin = np.linspace(0.0, 1.0, BS, dtype=np.float32)
    rs, cs = np.meshgrid(lin, lin, indexing="ij")
    zs = (rs + cs) / 2.0
    xs = np.stack([rs, cs, zs], -1).astype(np.float32)
    feats = []
    for p in [4.0 / (BS - 1), 16.0 / (BS - 1)]:
        a = (2.0 * np.pi * xs / p).astype(np.float32)
        feats.append(np.concatenate([np.cos(a), np.sin(a)], -1).astype(np.float32))
    return np.concatenate(feats, -1)  # (9, 9, 12)


def _prepare(obs, pos):
    single = obs[..., 0] - obs[..., 1]
    aug = np.pad(single, ((0, 0), (1, 1), (1, 1)))
    w = aug.shape[-1]
    outs = [aug[:, 1 + r : w - 1 + r, 1 + c : w - 1 + c] for (r, c) in OFFSETS]
    neigh = np.stack(outs, -1)
    n = obs.shape[0]
    stack = np.concatenate(
        [neigh, np.broadcast_to(pos, (n,) + pos.shape)], -1
    ).astype(np.float32)
    return stack.reshape(n, P, J)  # (B, 81, 19)


def _fold(inp):
    """Host-side constant folding of weights. All f32 numpy, unscaled."""
    scale = np.float32(1.0 / np.sqrt(D))
    Wk = inp["kvb_w"][:, :, :D]                                   # (L,19,512)
    Wv = inp["kvb_w"][:, :, D:]
    kvx_v = inp["kvx_w"][:, :, D:]                                # (L,512,512)
    qk_w = np.einsum("ldh,ljh->ldj", inp["q_w"], Wk) * scale      # (L,512,19)
    qk_b = np.einsum("lh,ljh->lj", inp["q_b"], Wk) * scale        # (L,19)
    afin = np.einsum("lde,leh->ldh", kvx_v, inp["fin_w"])         # (L,512,512)
    sfin = np.einsum("lje,leh->ljh", Wv, inp["fin_w"])            # (L,19,512)
    bias_v = inp["kvx_b"][:, D:] + inp["kvb_b"][:, D:]
    cfin = np.einsum("le,leh->lh", bias_v, inp["fin_w"]) + inp["fin_b"]
    return qk_w, qk_b, afin, sfin, cfin


def _ktile_lhsT(W):
    """(L,512,512) -> (L,128,2048) with col ((o*4+kt)*128+m) = W[l,kt*128+k,o*128+m]."""
    Lx = W.shape[0]
    return np.ascontiguousarray(
        W.reshape(Lx, 4, 128, 4, 128).transpose(0, 2, 3, 1, 4).reshape(Lx, 128, 2048)
    )


def _build_nc(alpha):
    nc = bacc.Bacc("TRN2", target_bir_lowering=False, debug=False)

    d_cpk16 = nc.dram_tensor("cpk16", [128, C16_END], bf16, kind="ExternalInput")
    d_e2pf = nc.dram_tensor("e2pf", [128, NB], f32, kind="ExternalInput")
    d_wbig = nc.dram_tensor("wbig", [L, 128, 3 * 2048], fp8, kind="ExternalInput")
    # per layer: qk k-tiles (4*19) then fq k-tiles (4*19)
    d_qkfq = nc.dram_tensor("qkfq", [128, L * 8 * J], bf16, kind="ExternalInput")
    d_sfall = nc.dram_tensor("sfall", [JP, L * D], bf16, kind="ExternalInput")
    d_bias = nc.dram_tensor("bias", [1, L * 2 * D], bf16, kind="ExternalInput")
    d_gconst = nc.dram_tensor("gconst", [1, L * J], f32, kind="ExternalInput")
    d_whead = nc.dram_tensor("whead", [128, 4 * POSD], bf16, kind="ExternalInput")
    d_out = nc.dram_tensor("out", [NB, P], f32, kind="ExternalOutput")

    def bcast_mid(ap2d, n):
        # (p, k) AP -> (p, n, k) with step-0 broadcast in the middle
        return bass.AP(
            tensor=ap2d.tensor, offset=ap2d.offset,
            ap=[ap2d.ap[0], [0, n], ap2d.ap[1]],
        )

    with tile.TileContext(nc) as tc:
        with (
            tc.tile_pool(name="consts", bufs=1) as consts,
            tc.tile_pool(name="wpool", bufs=8) as wpool,
            tc.tile_pool(name="ap", bufs=3) as apool,
            tc.tile_pool(name="attn", bufs=1) as atp,
            tc.tile_pool(name="pm", bufs=4, space="PSUM") as pm,
            tc.tile_pool(name="pt", bufs=2, space="PSUM") as pt,
        ):
            # ---- constants (7 DMAs) ----
            cpk16 = consts.tile([128, C16_END], bf16)
            nc.sync.dma_start(out=cpk16, in_=d_cpk16[:, :])
            e2pf = consts.tile([128, NB], f32)
            nc.sync.dma_start(out=e2pf, in_=d_e2pf[:, :])
            qkfq = consts.tile([128, L * 8 * J], bf16)
            nc.sync.dma_start(out=qkfq, in_=d_qkfq[:, :])
            sfall = consts.tile([JP, L * D], bf16)
            nc.sync.dma_start(out=sfall, in_=d_sfall[:, :])
            biasall = consts.tile([1, L * 2 * D], bf16)
            nc.sync.dma_start(out=biasall, in_=d_bias[:, :])
            gconst = consts.tile([1, L * J], f32)
            nc.sync.dma_start(out=gconst, in_=d_gconst[:, :])
            whead = consts.tile([128, 4 * POSD], bf16)
            nc.sync.dma_start(out=whead, in_=d_whead[:, :])

            bfpj4 = cpk16[:NP, C_BFPJ:C_BFJP].rearrange("p (a b) -> p a b", b=JP)
            bfjp4 = cpk16[:NP, C_BFJP:C_POST].rearrange("p (a b) -> p a b", b=PQP)
            wpost = cpk16[:POSD, C_POST:C_E2]     # (12, 81) bf16
            e2p = cpk16[:128, C_E2:C_E2T]         # (128, 32) bf16 group-major
            e2tp = cpk16[:NB, C_E2T:C16_END]      # (32, 128) bf16

            ones1 = consts.tile([1, NB], f32)
            nc.vector.memset(ones1, 1.0)
            ones_bf = consts.tile([1, NB], bf16)
            nc.vector.memset(ones_bf, 1.0)
            # persistent attention buffers (serial chain -> single-buffered)
            sT_buf = consts.tile([JP, NB], bf16)
            nc.vector.memset(sT_buf, 1.0)   # row 19 stays 1.0 (cfin ones row)
            g3 = consts.tile([NP, JP], bf16)
            nc.vector.memset(g3, 1.0)       # pad col 19 stays 1.0 (-40 lane)
            e4 = consts.tile([NP, PQP], bf16)
            nc.vector.memset(e4, 0.0)       # pad col 21 stays 0
            g_sb = atp.tile([NB, J], bf16, tag="gsb")
            s4 = atp.tile([NP, J], bf16, tag="s4")
            rs = atp.tile([NP, 1], f32, tag="rs")
            t1 = atp.tile([NP, PQ * JP], bf16, tag="t1")
            t2 = atp.tile([NP, J * PQP], bf16, tag="t2")
            dots = atp.tile([NP, PQ], f32, tag="dots")
            s4n = atp.tile([NP, J], bf16, tag="s4n")
            recip = atp.tile([NB, 1], bf16, tag="recip")

            # residual stream: x^T as (128, 4*32), pure bf16
            xT_b = apool.tile([128, 4 * NB], bf16, tag="xb")
            nc.vector.memset(xT_b, 0.0)
            xT_mid = xT_b

            def xsl(t, kt):
                return t[:, kt * NB : (kt + 1) * NB]

            t1_3 = t1[:, :].rearrange("p (a b) -> p a b", b=JP)
            t2_3 = t2[:, :].rearrange("p (a b) -> p a b", b=PQP)

            prev = None  # (wb, uT, y_ps, sign) of layer l-1 pending fc1
            for l in range(L):
                wb = wpool.tile([128, 3 * 2048], fp8, tag="wb")
                nc.sync.dma_start(out=wb, in_=d_wbig[l, :, :])

                def wtile(mat, o, kt, wbx=None):
                    wbx = wb if wbx is None else wbx
                    c = ((mat * 4 + o) * 4 + kt) * 128
                    return wbx[:, c : c + 128]

                qk_l = qkfq[:, l * 8 * J : l * 8 * J + 4 * J]
                fq_l = qkfq[:, l * 8 * J + 4 * J : (l + 1) * 8 * J]
                aab = abs(alpha[l])

                # ---- g (32,19) then replicate to 128 group-major rows ----
                g_ps = pt.tile([NB, J], f32, tag="sp")
                nc.tensor.matmul(
                    g_ps, ones1, gconst[:, l * J : (l + 1) * J],
                    start=True, stop=(l == 0),
                )
                if l > 0:
                    wb_p, uT_p, y_p, sgn_p = prev
                    for kt in range(4):
                        nc.tensor.matmul(
                            g_ps, xsl(xT_mid, kt),
                            qk_l[:, kt * J : (kt + 1) * J],
                            start=False, stop=False,
                        )
                    for kt in range(4):
                        nc.tensor.matmul(
                            g_ps, xsl(uT_p, kt),
                            fq_l[:, kt * J : (kt + 1) * J],
                            start=False, stop=(kt == 3),
                        )
                nc.vector.tensor_copy(g_sb, g_ps)
                g3_ps = pt.tile([NP, J], f32, tag="sp")
                nc.tensor.matmul(g3_ps, e2tp, g_sb, start=True, stop=True)
                nc.scalar.activation(g3[:, 0:J], g3_ps, Act.Copy)

                # ---- deferred fc1 of layer l-1 (hides under attention) ----
                if l > 0:
                    for o in range(4):
                        for kt in range(4):
                            nc.tensor.matmul(
                                xsl(y_p, o), wtile(2, o, kt, wb_p),
                                xsl(uT_p, kt),
                                start=False, stop=(kt == 3),
                            )

                # psum tiles + early bias rank-1s (no deps; run in PE slack)
                ft = pm.tile([128, 4 * NB], f32, tag="mm")
                u_ps = pm.tile([128, 4 * NB], f32, tag="mm")
                y_ps = pm.tile([128, 4 * NB], f32, tag="mm")
                for o in range(4):
                    nc.tensor.matmul(
                        xsl(u_ps, o),
                        biasall[:, l * 2 * D + o * 128 : l * 2 * D + (o + 1) * 128],
                        ones_bf, start=True, stop=False,
                    )
                    nc.tensor.matmul(
                        xsl(y_ps, o),
                        biasall[:, l * 2 * D + D + o * 128 : l * 2 * D + D + (o + 1) * 128],
                        ones_bf, start=True, stop=False,
                    )

                # ---- dots = bfeat . g -> (128, 21) ----
                nc.vector.tensor_tensor(
                    t1_3, bfpj4, bcast_mid(g3[:, :], PQ), op=AluOp.mult
                )
                nc.vector.tensor_reduce(
                    dots, t1_3, axis=mybir.AxisListType.X, op=AluOp.add
                )
                # fc1 residual of layer l-1 (DVE slot after dots, before t2)
                if l > 0:
                    nxb = apool.tile([128, 4 * NB], bf16, tag="xb")
                    nc.vector.tensor_tensor(
                        nxb, xT_mid, y_p,
                        op=AluOp.add if sgn_p >= 0 else AluOp.subtract,
                    )
                    xT_b = nxb
                    # fin + fc0 x-parts now that x_l is final
                    for o in range(4):
                        for kt in range(4):
                            nc.tensor.matmul(
                                xsl(ft, o), wtile(0, o, kt), xsl(xT_b, kt),
                                start=(kt == 0), stop=False,
                            )
                    for o in range(4):
                        for kt in range(4):
                            nc.tensor.matmul(
                                xsl(u_ps, o), wtile(1, o, kt), xsl(xT_b, kt),
                                start=False, stop=False,
                            )
                with nc.allow_low_precision(reason="softmax rowsum"):
                    nc.scalar.activation(
                        e4[:, 0:PQ], dots, Act.Exp, accum_out=rs
                    )
                rsb_ps = pt.tile([NB, 1], f32, tag="sp")
                nc.tensor.matmul(rsb_ps, e2pf, rs, start=True, stop=True)

                # ---- s4 = sum_p e[b,p] bfeat[b,p,j] (unnormalized) ----
                nc.vector.tensor_tensor(
                    t2_3, bfjp4, bcast_mid(e4[:, :], J), op=AluOp.mult
                )
                with nc.allow_low_precision(reason="softmax recip in bf16"):
                    nc.vector.reciprocal(recip, rsb_ps)
                r3_ps = pt.tile([128, 1], f32, tag="sp")
                nc.tensor.matmul(r3_ps, e2tp, recip, start=True, stop=True)
                with nc.allow_low_precision(reason="attention s in bf16"):
                    nc.vector.tensor_reduce(
                        s4, t2_3, axis=mybir.AxisListType.X, op=AluOp.add
                    )
                nc.vector.tensor_scalar_mul(s4n, s4, r3_ps[:, :])
                # group-sum to s^T directly: (19,32) = s4n.T @ e2
                sT_ps = pt.tile([J, NB], f32, tag="sp")
                nc.tensor.matmul(sT_ps, s4n, e2p, start=True, stop=True)
                nc.vector.tensor_copy(sT_buf[0:J, :], sT_ps)

                # ---- fin tail: += s @ [sfin;cfin] (K=20, bias inside) ----
                for o in range(4):
                    nc.tensor.matmul(
                        xsl(ft, o),
                        sfall[:, l * D + o * 128 : l * D + (o + 1) * 128],
                        sT_buf,
                        start=(l == 0), stop=True,
                    )
                # tv = alpha*relu(ft); fc0 tv-part rides on it so the
                # x_mid residual add leaves the critical chain
                tv = apool.tile([128, 4 * NB], bf16, tag="tv")
                nc.vector.tensor_scalar(
                    tv, ft, 0.0, float(alpha[l]), op0=AluOp.max, op1=AluOp.mult
                )
                for o in range(4):
                    for kt in range(4):
                        nc.tensor.matmul(
                            xsl(u_ps, o), wtile(1, o, kt), xsl(tv, kt),
                            start=False, stop=(kt == 3),
                        )
                nmid = apool.tile([128, 4 * NB], bf16, tag="xb")
                nc.vector.tensor_tensor(nmid, xT_b, tv, op=AluOp.add)
                xT_mid = nmid
                nuT = apool.tile([128, 4 * NB], bf16, tag="uT")
                nc.scalar.activation(nuT, u_ps, Act.Relu, scale=aab)
                uT = nuT
                # fc1 matmuls deferred to next layer (after its g matmuls)
                prev = (wb, uT, y_ps, 1.0 if alpha[l] >= 0 else -1.0)

            # flush last layer's fc1 + residual
            wb_p, uT_p, y_p, sgn_p = prev
            for o in range(4):
                for kt in range(4):
                    nc.tensor.matmul(
                        xsl(y_p, o), wtile(2, o, kt, wb_p), xsl(uT_p, kt),
                        start=False, stop=(kt == 3),
                    )
            nxb = apool.tile([128, 4 * NB], bf16, tag="xb")
            nc.vector.tensor_tensor(
                nxb, xT_mid, y_p,
                op=AluOp.add if sgn_p >= 0 else AluOp.subtract,
            )
            xT_b = nxb

            # ---- head: logits = log_softmax((x @ head_w) @ posT) ----
            zT_ps = pt.tile([POSD, NB], f32, tag="sp")
            for kt in range(4):
                nc.tensor.matmul(
                    zT_ps, whead[:, kt * POSD : (kt + 1) * POSD], xsl(xT_b, kt),
                    start=(kt == 0), stop=(kt == 3),
                )
            zT = apool.tile([POSD, NB], bf16, tag="zT")
            nc.vector.tensor_copy(zT, zT_ps)
            lg_ps = pt.tile([NB, P], f32, tag="sp")
            nc.tensor.matmul(lg_ps, zT, wpost, start=True, stop=True)
            lg = apool.tile([NB, P], f32, tag="lg")
            nc.scalar.activation(lg, lg_ps, Act.Copy)
            mx = apool.tile([NB, 1], f32, tag="mx")
            nc.vector.tensor_reduce(
                mx, lg[:, :], axis=mybir.AxisListType.X, op=AluOp.max
            )
            negmx = apool.tile([NB, 1], f32, tag="nmx")
            nc.vector.tensor_scalar_mul(negmx, mx, -1.0)
            ex = apool.tile([NB, P], f32, tag="ex")
            sume = apool.tile([NB, 1], f32, tag="sume")
            nc.scalar.activation(
                ex, lg, Act.Exp, bias=negmx[:, :], accum_out=sume
            )
            lse = apool.tile([NB, 1], f32, tag="lse")
            nc.scalar.activation(lse, sume, Act.Ln)
            c = apool.tile([NB, 1], f32, tag="c")
            nc.vector.tensor_add(c, mx, lse)
            outf = apool.tile([NB, P], f32, tag="outf")
            nc.vector.tensor_scalar(
                outf, lg[:, :], c[:, :], None, op0=AluOp.subtract
            )
            nc.sync.dma_start(out=d_out[:, :], in_=outf)

    nc.finalize()
    return nc


def kernel(**inputs):
    inp = {k: np.asarray(v, dtype=np.float32) for k, v in inputs.items()}
    pos = _positions()
    bfeat = _prepare(inp["obs"], pos)  # (256, 81, 19)
    qk_w, qk_b, afin, sfin, cfin = _fold(inp)
    alpha = inp["alpha"].astype(np.float32)

    wbig = np.concatenate(
        [_ktile_lhsT(afin), _ktile_lhsT(inp["fc0_w"]), _ktile_lhsT(inp["fc1_w"])],
        axis=2,
    ).astype(fp8_np)  # (L, 128, 6144)

    # g pipelining folds: fq_l = sign(a_{l-1}) * fc1_{l-1} @ qk_l
    fq = np.zeros((L, D, J), np.float32)
    gconst = qk_b.copy()
    for l in range(1, L):
        sgn = 1.0 if alpha[l - 1] >= 0 else -1.0
        fq[l] = sgn * (inp["fc1_w"][l - 1] @ qk_w[l])
        gconst[l] = qk_b[l] + alpha[l - 1] * (inp["fc1_b"][l - 1] @ qk_w[l])
    qkfq = np.zeros((128, L * 8 * J), np.float32)
    for l in range(L):
        qkfq[:, l * 8 * J : l * 8 * J + 4 * J] = (
            qk_w[l].reshape(4, 128, J).transpose(1, 0, 2).reshape(128, 4 * J)
        )
        qkfq[:, l * 8 * J + 4 * J : (l + 1) * 8 * J] = (
            fq[l].reshape(4, 128, J).transpose(1, 0, 2).reshape(128, 4 * J)
        )

    sfin_aug = np.concatenate([sfin, cfin[:, None, :]], axis=1)  # (L, 20, 512)
    sfall = np.ascontiguousarray(
        sfin_aug.transpose(1, 0, 2)
    ).reshape(JP, L * D).astype(bf16_np)

    biasall = np.zeros((1, L * 2 * D), np.float32)
    for l in range(L):
        biasall[0, l * 2 * D : l * 2 * D + D] = inp["fc0_b"][l]
        biasall[0, l * 2 * D + D : (l + 1) * 2 * D] = (
            (1.0 if alpha[l] >= 0 else -1.0) * alpha[l] * inp["fc1_b"][l]
        )
    gconst_v = gconst.reshape(1, L * J).astype(np.float32)
    whead = (
        inp["head_w"].reshape(4, 128, POSD).transpose(1, 0, 2)
        .reshape(128, 4 * POSD).astype(bf16_np)
    )

    # group-major constants: partition p = gg*32 + b, cell = gg*21 + pq
    e2 = np.zeros((NP, NB), np.float32)
    for gg in range(GG):
        for b in range(NB):
            e2[gg * NB + b, b] = 1.0

    in_maps = []
    for cc in range(NCORES):
        bf = bfeat[cc * NB : (cc + 1) * NB]          # (32, 81, 19)
        # pad cells 81..83 with zeros, cell c -> (gg=c//21, pq=c%21)
        bfp = np.zeros((NB, GG * PQ, J), np.float32)
        bfp[:, :P, :] = bf
        bfg = bfp.reshape(NB, GG, PQ, J).transpose(1, 0, 2, 3)  # (gg,b,pq,j)
        cpk = np.zeros((128, C16_END), np.float32)
        bfpj = np.zeros((GG, NB, PQ, JP), np.float32)
        bfpj[:, :, :, :J] = bfg
        # pad-cell kill switch: -40 in the j-pad lane (g3 pad col is 1.0)
        for c in range(P, GG * PQ):
            bfpj[c // PQ, :, c % PQ, J] = NEGBIG
        cpk[:, C_BFPJ:C_BFJP] = bfpj.reshape(NP, PQ * JP)
        bfjp = np.zeros((GG, NB, J, PQP), np.float32)
        bfjp[:, :, :, :PQ] = bfg.transpose(0, 1, 3, 2)
        cpk[:, C_BFJP:C_POST] = bfjp.reshape(NP, J * PQP)
        cpk[:POSD, C_POST:C_E2] = pos.reshape(P, POSD).T
        cpk[:, C_E2:C_E2T] = e2
        cpk[:NB, C_E2T:C16_END] = e2.T
        in_maps.append({
            "cpk16": cpk.astype(bf16_np), "e2pf": e2,
            "wbig": wbig, "qkfq": qkfq.astype(bf16_np), "sfall": sfall,
            "bias": biasall.astype(bf16_np), "gconst": gconst_v, "whead": whead,
        })

    nc = _build_nc([float(a) for a in alpha])
    res = run_bass_kernel_spmd(nc, in_maps, core_ids=list(range(NCORES)))
    out = np.concatenate([r["out"] for r in res.results], axis=0)  # (256, 81)
    return out.astype(np.float32)
